# revision 85
# baseline (speedup 1.0000x reference)
"""CSWin transformer block on 8 Trainium2 NeuronCores.

Data-parallel over batch: 32 images -> 4 images per core. Inside each core a
single fused Bass/Tile program runs, per image: LN1 -> qkv -> cross-shaped
window attention (+LePE depthwise conv) -> proj -> residual -> LN2 -> MLP ->
residual, with the MLP of image i interleaved into image i+1's attention.

Layout strategy:
  - residual stream is token-major fp16 ([128 token partitions, blocks*128
    feature cols]), per-image padded to 25 blocks of 128 tokens.
  - matmul-side activations are feature-major fp16 per image [128 ch, 3136].
  - PE transposes (fp16, via identity) bridge the two.
  - attention works on per-window quads: 4 (branch, head) groups row/col
    packed onto the PE array; softmax normalization deferred to after AV
    using 448-wide column sums (ones matmul) and a single fused divide.
  - LePE = 9 shifted matmuls against host-folded (Wv * tap-weight) matrices,
    accumulated in PSUM in the same window-major layout as the attention
    output; both are pushed through proj as K=64 partial matmuls.
"""

import numpy as np

B, RESO, DIM = 32, 56, 128
L = RESO * RESO            # 3136
NCORES = 8
IPC = B // NCORES          # images per core = 4
NT = IPC * L               # tokens per core = 12544
NBI = 25                   # padded 128-token blocks per image (24.5 -> 25)
IMGCOLS = NBI * 128        # 3200 token-major cols per image
WIN = 112                  # window size (56*2)
NW = 28                    # windows per branch per image
TW = 448                   # feature-major token tile (8 image rows)
NTILES = L // TW           # 7
HD_SCALE = float(32) ** -0.5

_CACHE = {}


def _prep_weights(inputs):
    f32 = np.float32
    f16 = np.float16
    g1 = np.asarray(inputs['norm1_g'], f32)
    b1 = np.asarray(inputs['norm1_b'], f32)
    qkv_w = np.asarray(inputs['qkv_w'], f32)
    wqkv = qkv_w * g1[:, None]
    bqkv = np.asarray(inputs['qkv_b'], f32) + b1 @ qkv_w
    assert np.max(np.abs(bqkv)) == 0.0, "nonzero qkv bias path not emitted"

    # lepe tap-folded v-projections: block (br, tap) -> wqkv_v[:,64br:+64]*w[tap,c]
    wv = wqkv[:, 256:384]
    lepe_w = [np.asarray(inputs['lepe_w0'], f32), np.asarray(inputs['lepe_w1'], f32)]
    wvtap = np.zeros((128, 18 * 64), f32)
    for br in range(2):
        for ki in range(3):
            for kj in range(3):
                tap = ki * 3 + kj
                wvtap[:, (br * 9 + tap) * 64:(br * 9 + tap + 1) * 64] = (
                    wv[:, br * 64:br * 64 + 64] * lepe_w[br][ki, kj, 0, :][None, :])

    wproj = np.asarray(inputs['proj_w'], f32)
    lepe_b = np.concatenate([np.asarray(inputs['lepe_b0'], f32),
                             np.asarray(inputs['lepe_b1'], f32)])
    projb = np.asarray(inputs['proj_b'], f32) + lepe_b @ wproj

    g2 = np.asarray(inputs['norm2_g'], f32)
    b2 = np.asarray(inputs['norm2_b'], f32)
    fc1_w = np.asarray(inputs['fc1_w'], f32)
    wfc1 = fc1_w * g2[:, None]
    fc1b = np.asarray(inputs['fc1_b'], f32) + b2 @ fc1_w
    wfc2 = np.asarray(inputs['fc2_w'], f32)
    fc2b = np.asarray(inputs['fc2_b'], f32)

    # fp8 DoubleRow lepe weights folded through proj, x S8 scale (undone on
    # the att_f evac). Each MM slot is a [128(xhat ch), 2(pair), 128(proj
    # out)] stationary; pair element B may be zero (padding pair).
    import ml_dtypes
    S8 = 4096.0

    def wt(br, ki, kj):
        t = ki * 3 + kj
        return (wvtap[:, (br * 9 + t) * 64:(br * 9 + t + 1) * 64]
                @ wproj[64 * br:64 * br + 64, :]) * S8

    zz = np.zeros((128, 128), f32)
    slots = [
        (wt(0, 0, 1), wt(0, 2, 1)),   # 1  b0 row-pair, dj=0, full
        (wt(0, 0, 2), wt(0, 2, 2)),   # 2  b0 row-pair, dj=+1, even cols
        (wt(0, 0, 0), wt(0, 2, 0)),   # 3  b0 row-pair, dj=-1, odd cols
        (wt(0, 1, 1), zz),            # 4  b0 center, full
        (wt(0, 1, 2), zz),            # 5  b0 kj=+1, even cols
        (wt(0, 1, 0), zz),            # 6  b0 kj=-1, odd cols
        (wt(1, 1, 0), wt(1, 1, 2)),   # 7  b1 dj-pair, dii=0, full
        (wt(1, 2, 0), wt(1, 2, 2)),   # 8  b1 diag, dii=+1, even rows
        (wt(1, 0, 0), wt(1, 0, 2)),   # 9  b1 diag, dii=-1, odd rows
        (wt(1, 1, 1), wt(1, 2, 1)),   # 10 b1 (center,ki=+1), even rows
        (wt(1, 0, 1), wt(1, 1, 1)),   # 11 b1 (ki=-1,center), odd rows
    ]
    wlep8 = np.concatenate([np.concatenate(s, 1) for s in slots], 1)
    assert np.abs(wlep8).max() < 400.0, np.abs(wlep8).max()
    wprojh = np.concatenate([
        np.concatenate([wproj[0:64], np.zeros((64, 128), f32)], 0),
        np.concatenate([np.zeros((64, 128), f32), wproj[64:128]], 0)],
        1) * S8
    assert np.abs(wprojh).max() < 6e4

    # fp8 DoubleRow branch-1 qkv / v / fc1 weights: K=128 split into
    # channel pairs (c, c+64) matching the xh8p paired-activation layout.
    # x32 scale keeps fp8 values out of the subnormal range; undone at the
    # exp (q*k -> /1024), the v evac (/32) and the gelu (/32).
    SQ = 32.0

    def pair_k(w):  # [128, M] -> [64, 2, M] -> [64, 2*M]
        return np.concatenate([w[0:64], w[64:128]], 1)

    wqk8 = np.concatenate([
        np.concatenate([np.zeros((64, 64), f32), wqkv[0:64, 64:128] * SQ,
                        np.zeros((64, 64), f32), wqkv[64:128, 64:128] * SQ],
                       1),
        np.concatenate([np.zeros((64, 64), f32), wqkv[0:64, 192:256] * SQ,
                        np.zeros((64, 64), f32), wqkv[64:128, 192:256] * SQ],
                       1)], 1)  # [64, 512]: q-b1 pad128-pair, k-b1 pad128-pair
    wv8 = pair_k(wqkv[:, 320:384] * SQ)       # [64, 128]
    wfc18 = np.concatenate(                   # [64, 1024], per-chunk pairs
        [pair_k(wfc1[:, m * 128:(m + 1) * 128] * SQ) for m in range(4)], 1)
    for nm, arr in (("wqk8", wqk8), ("wv8", wv8), ("wfc18", wfc18)):
        assert np.abs(arr).max() < 400.0, (nm, np.abs(arr).max())

    return {
        'wqkv': wqkv.astype(f16),
        'wqk8': np.ascontiguousarray(wqk8).astype(ml_dtypes.float8_e4m3),
        'wv8': np.ascontiguousarray(wv8).astype(ml_dtypes.float8_e4m3),
        'wfc18': np.ascontiguousarray(wfc18).astype(ml_dtypes.float8_e4m3),
        'wlep8': np.ascontiguousarray(wlep8).astype(ml_dtypes.float8_e4m3),
        'wprojh': np.ascontiguousarray(wprojh).astype(f16),
        'projb': np.ascontiguousarray(projb.reshape(128, 1)),
        'wfc1': wfc1.astype(f16),
        'fc1b': np.ascontiguousarray(fc1b.reshape(4, 128).T),
        'wfc2': np.ascontiguousarray(
            (wfc2.reshape(4, 128, 128).transpose(1, 0, 2).reshape(128, 512))
            * 16.0).astype(__import__('ml_dtypes').float8_e4m3),
        'fc2b': np.ascontiguousarray(fc2b.reshape(128, 1)),
    }


def build_program(stop_after=None):
    import concourse.tile as tile
    from concourse import bacc, mybir

    dt = mybir.dt

    nc = bacc.Bacc("TRN2", target_bir_lowering=False, debug=False,
                   num_devices=NCORES)

    xin = nc.dram_tensor("xin", [NT, DIM], dt.float32, kind="ExternalInput").ap()
    wd = {}
    for name, shape, d in [
            ("wqkv", [128, 384], dt.float16),
            ("wqk8", [64, 512], dt.float8e4),
            ("wv8", [64, 128], dt.float8e4),
            ("wfc18", [64, 1024], dt.float8e4),
            ("wlep8", [128, 2816], dt.float8e4),
            ("wprojh", [128, 256], dt.float16), ("projb", [128, 1], dt.float32),
            ("wfc1", [128, 512], dt.float16), ("fc1b", [128, 4], dt.float32),
            ("wfc2", [128, 512], dt.float8e4),
            ("fc2b", [128, 1], dt.float32)]:
        wd[name] = nc.dram_tensor(name, shape, d, kind="ExternalInput").ap()
    out_d = nc.dram_tensor("out", [NT, DIM], dt.float32, kind="ExternalOutput").ap()

    with tile.TileContext(nc) as tc:
        _body(tc, mybir, xin, out_d, wd, stop_after=stop_after)
    nc.compile()
    return nc


def _body(tc, mybir, xin, out_d, wd, stop_after=None):
    nc = tc.nc
    dt = mybir.dt
    AF = mybir.ActivationFunctionType
    OP = mybir.AluOpType
    f16, f32 = dt.float16, dt.float32

    dumped = []

    def dump(t, ncols):
        """debug: DMA first min(ncols,12544) cols of [128,*] tile to out."""
        n = min(ncols, NT) // 128 * 128
        dv = out_d[0:n, :].rearrange("(b p) f -> p b f", p=128)
        nc.gpsimd.dma_start(dv, t[:, 0:n].rearrange("p (b f) -> p b f",
                                                    f=128))
        dumped.append(True)

    STAGES = ["load", "ln1", "t1", "qkv", "v", "lepe", "qk", "exp", "sums",
              "av", "recip", "quads", "proj", "ln2", "t2", "mlp"]
    lim = STAGES.index(stop_after) if stop_after else len(STAGES)

    def go(stage):
        return STAGES.index(stage) <= lim

    # ---------------- persistent pools ----------------
    wpool = tc.alloc_tile_pool(name="weights", bufs=1)
    gpool = tc.alloc_tile_pool(name="globals", bufs=1)

    wqkv = wpool.tile([128, 384], f16)
    wlep8 = wpool.tile([128, 2816], dt.float8e4)
    wprojh = wpool.tile([128, 256], f16)
    projb = wpool.tile([128, 1], f32)
    wfc1 = wpool.tile([128, 512], f16)
    fc1b = wpool.tile([128, 4], f32)
    wfc2 = wpool.tile([128, 512], dt.float8e4)
    fc2b = wpool.tile([128, 1], f32)
    ones_t = wpool.tile([128, 32], f16)
    epsb = wpool.tile([128, 1], f32)
    nc.vector.memset(epsb[:], 1e-5)
    # weight DMAs sequenced after the first image-load chunk on the (single)
    # DMA resource: they are not needed until the first qkv matmul
    with tc.tile_wait_until(0.005):
        for name, t in [("wqkv", wqkv), ("wlep8", wlep8),
                        ("wprojh", wprojh),
                        ("projb", projb), ("wfc1", wfc1), ("fc1b", fc1b),
                        ("wfc2", wfc2), ("fc2b", fc2b)]:
            nc.sync.dma_start(t[:], wd[name])
    nc.vector.memset(ones_t[:], 1.0)

    # token-major global tensors, per-image padded to 25 blocks.
    # xh_tm is a 2-image ring: each slice is written by an LN and read by
    # the immediately following transpose.
    x_tm = gpool.tile([128, IPC * IMGCOLS], f16)
    xh_tm = gpool.tile([128, 2 * IMGCOLS], f16)
    r1_tm = x_tm  # residual accumulates in place

    # fp8 natural-order xhat with 56-col zero pads top/bottom (+extra tail
    # pad so zero-weight padding pairs stay in bounds); 2-image ring,
    # per-image data DMA, pads memset once.
    xq8r = gpool.tile([128, 2 * 3360], dt.float8e4)
    for s in range(2):
        nc.gpsimd.memset(xq8r[:, s * 3360:s * 3360 + 56], 0.0)
        nc.gpsimd.memset(xq8r[:, s * 3360 + 3192:s * 3360 + 3360], 0.0)

    import bass_rust as _br

    def ap_pair(t_ap, off1, off2, n):
        """[128, 2, n] overlapping view: pair elements at cols off1/off2."""
        v = t_ap[:, off1:off1 + n].unsqueeze(1)
        raw = [list(x) for x in v.ap]
        raw[1] = [off2 - off1, 2]
        c = v.copy()
        c.ap = _br.VecI64Pair(raw)
        return c

    ATT_SCALE = 1.0 / 4096.0

    NBLK = IPC * NBI  # 100 token blocks

    # ---------------- per-image load (cast fp32->fp16), chunked ----------
    def load_img_chunk(i, b0, b1):
        src = xin[i * L:(i + 1) * L, :]
        bf = min(b1, 24)
        if bf > b0:
            full = src[b0 * 128:bf * 128, :].rearrange("(b p) f -> p b f", p=128)
            dst = x_tm[:, i * IMGCOLS + b0 * 128:i * IMGCOLS + bf * 128
                       ].rearrange("p (b f) -> p b f", b=bf - b0)
            nc.gpsimd.dma_start(dst, full)
        if b1 == NBI:
            nc.gpsimd.dma_start(
                x_tm[0:64, i * IMGCOLS + 24 * 128:i * IMGCOLS + 25 * 128],
                src[24 * 128:L, :])
            nc.gpsimd.memset(
                x_tm[64:128, i * IMGCOLS + 24 * 128:i * IMGCOLS + 25 * 128], 0.0)

    def layernorm(src_tm, dst_tm, sname, blk0, dst_blk0, chunks,
                  pre_chunk=None, post_chunk=None, newton2=False,
                  split_merge=False, apply_pool=False):
        """token-major LN over NBI blocks at blk0, processed in chunks:
        bn_stats per chunk (after pre_chunk), ONE even/odd merge + rsqrt
        (quadratic seed + Newton on DVE), then fused apply per chunk
        followed by post_chunk (e.g. a transpose). split_merge runs the
        whole pipeline per chunk (shorter latency, more DVE ops)."""
        if split_merge:
            for b0, b1 in chunks:
                layernorm(src_tm, dst_tm, f"{sname}_{b0}", blk0, dst_blk0,
                          ((b0, b1),), pre_chunk, post_chunk, newton2,
                          apply_pool=apply_pool)
            return
        nb = NBI
        with tc.tile_pool(name=f"ln_{sname}", bufs=1) as lp:
            st6 = lp.tile([128, nb * 8], f32, name=f"st6_{sname}")
            rs_t = lp.tile([128, nb], f32, name=f"rs_{sname}")
            c_t = lp.tile([128, nb], f32, name=f"c_{sname}")
            mu_t = lp.tile([128, nb], f32, name=f"mu_{sname}")
            m2_t = lp.tile([128, nb], f32, name=f"m2_{sname}")
            for b0, b1 in chunks:
                if pre_chunk is not None:
                    pre_chunk(b0, b1)
                for b in range(b0, b1):
                    nc.vector.bn_stats(st6[:, b * 8:b * 8 + 6],
                                       src_tm[:, (blk0 + b) * 128:
                                              (blk0 + b + 1) * 128])
            c0, c1 = chunks[0][0], chunks[-1][1]
            sp = slice(c0, c1)
            sv = st6.rearrange("p (b s) -> p b s", s=8)
            # stats6 = [cnt_e, mean_e, cnt*var_e, cnt_o, mean_o, cnt*var_o]
            nc.vector.tensor_tensor(mu_t[:, sp], sv[:, sp, 1:2], sv[:, sp, 4:5], OP.add)
            nc.vector.tensor_tensor(c_t[:, sp], sv[:, sp, 1:2], sv[:, sp, 4:5], OP.subtract)
            nc.vector.tensor_tensor(c_t[:, sp], c_t[:, sp], c_t[:, sp], OP.mult)
            nc.vector.tensor_tensor(m2_t[:, sp], sv[:, sp, 2:3], sv[:, sp, 5:6], OP.add)
            nc.vector.scalar_tensor_tensor(m2_t[:, sp], c_t[:, sp], 32.0, m2_t[:, sp],
                                           OP.mult, OP.add)
            t_t = lp.tile([128, nb], f32, name=f"t_{sname}")
            nc.vector.tensor_scalar(m2_t[:, sp], m2_t[:, sp], 1.0 / 128.0, 1e-5,
                                    OP.mult, OP.add)
            nc.vector.tensor_scalar(rs_t[:, sp], m2_t[:, sp], 0.23645927242441878,
                                    -1.0257861053814088, OP.mult, OP.add)
            nc.vector.tensor_tensor(rs_t[:, sp], rs_t[:, sp], m2_t[:, sp], OP.mult)
            nc.vector.tensor_scalar(rs_t[:, sp], rs_t[:, sp], 1.8125565144482214,
                                    None, OP.add)
            for _ in range(2 if newton2 else 1):
                nc.vector.tensor_tensor(t_t[:, sp], rs_t[:, sp], rs_t[:, sp], OP.mult)
                nc.vector.tensor_tensor(t_t[:, sp], t_t[:, sp], m2_t[:, sp], OP.mult)
                nc.vector.tensor_scalar(t_t[:, sp], t_t[:, sp], -0.5, 1.5,
                                        OP.mult, OP.add)
                nc.vector.tensor_tensor(rs_t[:, sp], rs_t[:, sp], t_t[:, sp], OP.mult)
            # c = -(mu_sum/2)*rs
            nc.vector.scalar_tensor_tensor(c_t[:, sp], mu_t[:, sp], -0.5, rs_t[:, sp],
                                           OP.mult, OP.mult)
            aeng = nc.gpsimd if apply_pool else nc.vector
            for b0, b1 in chunks:
                for b in range(b0, b1):
                    aeng.tensor_scalar(
                        dst_tm[:, (dst_blk0 + b) * 128:(dst_blk0 + b + 1) * 128],
                        src_tm[:, (blk0 + b) * 128:(blk0 + b + 1) * 128],
                        rs_t[:, b:b + 1], c_t[:, b:b + 1], OP.mult, OP.add)
                if post_chunk is not None:
                    post_chunk(b0, b1)

    def transpose_chunk(dst, src_tm, slot, b0, b1):
        """xbar DMA transpose of token-major blocks [b0:b1] of ring slot into
        feature-major dst cols [b0*128:b1*128)."""
        nc.sync.dma_start_transpose(
            dst[:, b0 * 128:b1 * 128].rearrange("p (b f) -> p b f", b=b1 - b0),
            src_tm[:, slot * IMGCOLS + b0 * 128:slot * IMGCOLS + b1 * 128])

    # ---------------- fused per-image loop ----------------
    with tc.tile_pool(name="att_fm", bufs=1) as ap, \
         tc.tile_pool(name="att_ps", bufs=1, space="PSUM") as app, \
         tc.tile_pool(name="att_sb", bufs=3) as asb, \
         tc.tile_pool(name="mlp_fm", bufs=1) as mfm, \
         tc.tile_pool(name="mlp_sb", bufs=2) as msb:
        xh_tiles = {}
        xh2_tiles = {}
        mlp_states = {}

        def epi_chunk(i, mlp_f, b0, b1):
            """transpose + residual + store of output blocks [b0:b1).
            Non-tail images add in fp16 and let the (gpsimd) store DMA
            upcast — halves the DVE cost of the residual adds; the last
            image keeps fp32 + HWDGE stores for the shortest drain."""
            nb = b1 - b0
            fin = True
            mlp_t = mfm.tile([128, 1024], f16, name="mlp_t", tag="mlp_t",
                             bufs=2)
            o_sb = mfm.tile([128, 1024], f32, name="o_sb",
                            tag="o_sbF", bufs=2)
            nc.sync.dma_start_transpose(
                mlp_t[:, 0:nb * 128].rearrange("p (b f) -> p b f", b=nb),
                mlp_f[:, b0 * 128:b1 * 128])
            nc.vector.tensor_tensor(
                o_sb[:, 0:nb * 128], mlp_t[:, 0:nb * 128],
                r1_tm[:, i * IMGCOLS + b0 * 128:i * IMGCOLS + b1 * 128],
                OP.add)
            eng = nc.sync if fin else nc.gpsimd
            nfull = min(b1, 24) - b0
            if nfull > 0:
                dst = out_d[i * L + b0 * 128:i * L + (b0 + nfull) * 128, :
                            ].rearrange("(b p) f -> p b f", p=128)
                eng.dma_start(dst, o_sb[:, 0:nfull * 128].rearrange(
                    "p (b f) -> p b f", b=nfull))
            if b1 == 25:
                # tail: blocks 24..24.5 (64 tokens)
                eng.dma_start(out_d[i * L + 24 * 128:(i + 1) * L, :],
                              o_sb[0:64, nfull * 128:nfull * 128 + 128])

        def mlp_start(i):
            xh2 = xh2_tiles.pop(i)
            xh28p = None
            return {
                'i': i,
                'xh2': xh2,
                'xh28p': xh28p,
                'mlp_f': mfm.tile([128, IMGCOLS], f16, name="mlp_f",
                                  tag="mlp_f", bufs=2),
                'h2': {},
            }

        def mlp_fc1(st, t):
            rhs = st['xh2'][:, t * TW:(t + 1) * TW]
            rhs8 = None
            if st['xh28p'] is not None:
                rhs8 = st['xh28p'].rearrange("p (two c) -> p two c", two=2)[
                    :, :, t * TW:(t + 1) * TW]
            h2s = []
            for half in range(2):
                h2 = msb.tile([128, 2 * TW], dt.float8e4,
                              name=f"h2_{half}", tag="hs", bufs=8)
                ps_h = app.tile([128, 1024], f32, name=f"ps_h{half}",
                                tag="pQK", bufs=2)
                for mm in range(2):
                    m = 2 * half + mm
                    if rhs8 is not None:
                        nc.tensor.matmul(
                            ps_h[:, mm * 512:mm * 512 + TW],
                            wfc18[:, m * 256:(m + 1) * 256].rearrange(
                                "p (two c) -> p two c", two=2),
                            rhs8, start=True, stop=True,
                            perf_mode=mybir.MatmulPerfMode.DoubleRow,
                            skip_group_check=True)
                    else:
                        nc.tensor.matmul(ps_h[:, mm * 512:mm * 512 + TW],
                                         wfc1[:, m * 128:(m + 1) * 128],
                                         rhs, start=True, stop=True,
                                         skip_group_check=True)
                nc.scalar.activation(
                    h2.rearrange("p (m c) -> p m c", m=2),
                    ps_h.rearrange("p (m c) -> p m c", m=2)[:, :, 0:TW],
                    AF.Gelu,
                    scale=(1.0 / 32.0) if rhs8 is not None else 1.0)
                h2s.append(h2)
            st['h2'][t] = h2s

        def mlp_fc2(st, t):
            i = st['i']
            mlp_f = st['mlp_f']
            h2s = st['h2'].pop(t)
            ps_o = app.tile([128, 512], f32, name="ps_o", tag="pC", bufs=2)
            # fc2: fp8 DoubleRow pairs two K=128 chunks per pass (weights
            # host-scaled x16; undone in the bias add below)
            for half in range(2):
                nc.tensor.matmul(
                    ps_o[:, 0:TW],
                    wfc2[:, half * 256:(half + 1) * 256].rearrange(
                        "p (two m) -> p two m", two=2),
                    h2s[half].rearrange("p (two c) -> p two c", two=2),
                    start=(half == 0), stop=(half == 1),
                    perf_mode=mybir.MatmulPerfMode.DoubleRow,
                    skip_group_check=True)
            nc.vector.tensor_scalar(mlp_f[:, t * TW:(t + 1) * TW],
                                    ps_o[:, 0:TW], 1.0 / 16.0,
                                    fc2b[:, 0:1], OP.mult, OP.add)
            # stream the epilogue as its input columns land
            if t == 2:
                epi_chunk(i, mlp_f, 0, 8)
            elif t == 4:
                epi_chunk(i, mlp_f, 8, 16)
            elif t == 5:
                epi_chunk(i, mlp_f, 16, 20)
            elif t == 6:
                epi_chunk(i, mlp_f, 20, 24)
                nc.vector.memset(mlp_f[:, L:IMGCOLS], 0.0)
                epi_chunk(i, mlp_f, 24, 25)

        def do_mlp_tail(st):
            if st['xh28p'] is not None:
                xh2 = st['xh2']
                nc.gpsimd.dma_start(st['xh28p'][:, 0:IMGCOLS], xh2[0:64, :])
                nc.gpsimd.dma_start(st['xh28p'][:, IMGCOLS:2 * IMGCOLS],
                                    xh2[64:128, :])
            for t in range(NTILES):
                mlp_fc1(st, t)
                if t >= 1:
                    mlp_fc2(st, t - 1)
            mlp_fc2(st, NTILES - 1)

        def stage_in(i):
            xh = ap.tile([128, IMGCOLS], f16, name="xh_fm", tag="xh", bufs=2)
            slot = i % 2

            def load(b0, b1):
                # keep prefetched images' loads well out of the startup and
                # per-image copy windows: they pollute the rotating DMA
                # semaphores and hog the DMA engines otherwise
                with tc.tile_wait_until(0.06 * (i - 1), enable=i >= 2):
                    load_img_chunk(i, b0, b1)

            layernorm(x_tm, xh_tm, f"ln1_{i}", blk0=NBI * i,
                      dst_blk0=NBI * slot, chunks=((0, 13), (13, NBI)),
                      pre_chunk=load,
                      post_chunk=lambda b0, b1: transpose_chunk(
                          xh, xh_tm, slot, b0, b1),
                      split_merge=(i < 2))
            xh_tiles[i] = xh

        stage_in(0)
        stage_in(1)
        for i in range(IPC):
            xh = xh_tiles[i]
            # fp8 copies of xhat for the DoubleRow lepe taps, issued first:
            # natural-order padded xq8 + the (j-1, j+1) edge-zeroed shifted
            # pair xb1p. (Consumed only at the lepe+proj stage — a full
            # image of slack.)
            xq8 = xq8r[:, (i % 2) * 3360:(i % 2) * 3360 + 3360]
            xb1p = ap.tile([128, 2 * L], dt.float8e4, name="xb1p", bufs=2)
            fp16_img = True
            nc.gpsimd.dma_start(xq8[:, 56:3192], xh[:, 0:L])
            nc.gpsimd.memset(xb1p[:, 0:1], 0.0)
            nc.gpsimd.memset(xb1p[:, 2 * L - 1:2 * L], 0.0)
            nc.gpsimd.dma_start(xb1p[:, 1:L], xh[:, 0:L - 1])
            nc.gpsimd.dma_start(xb1p[:, L:2 * L - 1], xh[:, 1:L])
            xb1e = xb1p.rearrange("p (c i j) -> p c i j", c=2, j=RESO)
            nc.gpsimd.memset(xb1e[:, 0, :, 0:1], 0.0)
            nc.gpsimd.memset(xb1e[:, 1, :, 55:56], 0.0)
            # q/k storage: rows 0:64 (branch0) in window-major cols
            # (112w + 2i + jj), rows 64:128 (branch1) natural (= window-major)
            qk_wm = ap.tile([128, 2 * L], f16, name="qk_wm", bufs=2)
            q_wm = qk_wm[:, 0:L]
            k_wm = qk_wm[:, L:2 * L]
            xh_b0wm = ap.tile([128, L], f16, name="xh_b0wm", bufs=1)
            v_wm = ap.tile([128, 2 * NW * 64], f16, name="v_wm", bufs=1)
            o_wm = ap.tile([128, L], f16, name="o_wm")
            att_f = ap.tile([128, IMGCOLS], f16, name="att_f")

            # qkv: q,k. branch0 halves computed directly in window-major
            # order by a window-ordered moving-operand gather; branch1
            # window-major == natural. One paired evac per tile.
            xq0 = xh[:, 0:L].rearrange("p (i w jj) -> p w i jj", w=NW, jj=2)
            qkv = qk_wm.rearrange("p (qk c) -> p qk c", qk=2)
            for t in range(NTILES) if go("qkv") else []:
                ps_qk = app.tile([128, 1024], f32, name="ps_qk", tag="pQ2",
                                 bufs=1)
                ps_q = ps_qk[:, 0:512]
                ps_k = ps_qk[:, 512:1024]
                DRm = mybir.MatmulPerfMode.DoubleRow
                rhs_wm = xq0[:, 4 * t:4 * t + 4, :, :]
                if fp16_img:
                    # startup/tail image: fp16 branch1 (no xh8p dependency)
                    rhs_nat = xh[:, t * TW:(t + 1) * TW]
                    nc.tensor.matmul(ps_q[64:128, 0:TW], wqkv[:, 64:128],
                                     rhs_nat, start=True, stop=True,
                                     tile_position=(0, 64),
                                     skip_group_check=True)
                    nc.tensor.matmul(ps_q[0:64, 0:TW], wqkv[:, 0:64], rhs_wm,
                                     start=True, stop=True,
                                     tile_position=(0, 0),
                                     skip_group_check=True)
                    nc.tensor.matmul(ps_k[64:128, 0:TW], wqkv[:, 192:256],
                                     rhs_nat, start=True, stop=True,
                                     tile_position=(0, 64),
                                     skip_group_check=True)
                    nc.tensor.matmul(ps_k[0:64, 0:TW], wqkv[:, 128:192],
                                     rhs_wm, start=True, stop=True,
                                     tile_position=(0, 0),
                                     skip_group_check=True)
                else:
                    rhs_nat8 = xh8p.rearrange("p (two c) -> p two c", two=2)[
                        :, :, t * TW:(t + 1) * TW]
                    # branch1 halves via fp8 DoubleRow (zero-padded to dst
                    # base 0, writes all 128 rows), then branch0 fp16 accums
                    nc.tensor.matmul(ps_q[:, 0:TW],
                                     wqk8[:, 0:256].rearrange(
                                         "p (two m) -> p two m", two=2),
                                     rhs_nat8, start=True, stop=False,
                                     perf_mode=DRm, tile_position=(0, 0),
                                     skip_group_check=True)
                    nc.tensor.matmul(ps_q[0:64, 0:TW], wqkv[:, 0:64], rhs_wm,
                                     start=False, stop=True,
                                     tile_position=(0, 0),
                                     skip_group_check=True)
                    nc.tensor.matmul(ps_k[:, 0:TW],
                                     wqk8[:, 256:512].rearrange(
                                         "p (two m) -> p two m", two=2),
                                     rhs_nat8, start=True, stop=False,
                                     perf_mode=DRm, tile_position=(0, 0),
                                     skip_group_check=True)
                    nc.tensor.matmul(ps_k[0:64, 0:TW], wqkv[:, 128:192],
                                     rhs_wm, start=False, stop=True,
                                     tile_position=(0, 0),
                                     skip_group_check=True)
                # split evac: k on DVE, q on Act — the single psum buffer
                # frees as fast as the next tile's matmuls need it
                nc.vector.tensor_copy(k_wm[:, t * TW:(t + 1) * TW],
                                      ps_qk[:, 512:512 + TW])
                nc.scalar.copy(q_wm[:, t * TW:(t + 1) * TW],
                               ps_qk[:, 0:TW])

            # window-major xhat copy for branch-0 stationary operands.
            # The last image's copy runs on DVE: Pool's queue is deep with
            # LN applies and fp8 copies right then, and v-b0 would stall.
            geng = nc.gpsimd
            xb0 = xh_b0wm.rearrange("p (w q2 jj) -> p q2 w jj", w=NW, q2=RESO)
            for t in range(NTILES) if go("v") else []:
                geng.tensor_copy(xb0[:, 8 * t:8 * t + 8, :, :],
                                 xh[:, t * TW:(t + 1) * TW])

            # v window-major; branch1 via fp8 DoubleRow (x32 weights,
            # undone at the evac)
            for br in range(2) if go("v") else []:
                for wg in range(NW // 4):
                    ps_v = app.tile([128, 512], f32, name="ps_v", tag="pC",
                                    bufs=2)
                    for wi in range(4):
                        w = wg * 4 + wi
                        if br == 0:
                            nc.tensor.matmul(
                                ps_v[0:WIN, wi * 64:wi * 64 + 64],
                                xh_b0wm[:, WIN * w:WIN * w + WIN],
                                wqkv[:, 256:320], start=True, stop=True)
                        elif fp16_img:
                            nc.tensor.matmul(
                                ps_v[0:WIN, wi * 64:wi * 64 + 64],
                                xh[:, WIN * w:WIN * w + WIN],
                                wqkv[:, 320:384], start=True, stop=True)
                        else:
                            nc.tensor.matmul(
                                ps_v[0:WIN, wi * 64:wi * 64 + 64],
                                xh8v[:, :, WIN * w:WIN * w + WIN],
                                wv8v, start=True, stop=True,
                                perf_mode=mybir.MatmulPerfMode.DoubleRow)
                    if br == 0:
                        nc.scalar.copy(
                            v_wm[0:WIN, (wg * 4) * 64:(wg * 4 + 4) * 64],
                            ps_v[0:WIN, 0:256])
                    elif fp16_img:
                        nc.scalar.copy(
                            v_wm[0:WIN, (NW + wg * 4) * 64:(NW + wg * 4 + 4) * 64],
                            ps_v[0:WIN, 0:256])
                    else:
                        nc.scalar.activation(
                            v_wm[0:WIN, (NW + wg * 4) * 64:(NW + wg * 4 + 4) * 64],
                            ps_v[0:WIN, 0:256], AF.Copy, scale=1.0 / 32.0)

            def lepe_tile(t, ps_p):
                """9-tap depthwise LePE conv x Wproj, natural token order,
                accumulated into the proj psum: 11 fp8 DoubleRow MMs with
                parity masking via strided psum writes."""
                DR = mybir.MatmulPerfMode.DoubleRow
                T0 = TW * t
                b = 56 + T0  # xq8 col of token T0

                def wsl(k):
                    return wlep8[:, k * 256:k * 256 + 256].rearrange(
                        "p (two m) -> p two m", two=2)

                def mm(k, mov, out, start=False):
                    nc.tensor.matmul(out, wsl(k), mov, start=start,
                                     stop=False, perf_mode=DR,
                                     tile_position=(0, 0),
                                     skip_group_check=True)

                psF = ps_p[:, 0:TW]
                psc = ps_p[:, 0:TW].rearrange("p (c s) -> p c s", s=2)
                psE, psO = psc[:, :, 0:1], psc[:, :, 1:2]
                psr = ps_p[:, 0:TW].rearrange("p (r c) -> p r c", c=56)
                psER, psOR = psr[:, 0:8:2, :], psr[:, 1:8:2, :]

                def ev(pair):  # stride-2 moving view (base = first element)
                    return pair.rearrange("p two (c s) -> p two c s",
                                          s=2)[:, :, :, 0:1]

                def er(pair):  # every-other-56-block moving view
                    return pair.rearrange("p two (r c) -> p two r c",
                                          c=56)[:, :, 0:8:2, :]

                # branch 0 (vertical stripes)
                mm(0, ap_pair(xq8, b - 56, b + 56, TW), psF, start=True)
                mm(1, ev(ap_pair(xq8, b - 55, b + 57, TW)), psE)
                mm(2, ev(ap_pair(xq8, b - 56, b + 56, TW)), psO)
                mm(3, ap_pair(xq8, b, b + 112, TW), psF)
                mm(4, ev(ap_pair(xq8, b + 1, b + 113, TW)), psE)
                mm(5, ev(ap_pair(xq8, b, b + 112, TW)), psO)
                # branch 1 (horizontal stripes); xb1p halves: h0[c]=x[c-1]
                # (kj=0 tap), h1[c]=x[c+1] (kj=2), j-edge zeroed
                mm(6, ap_pair(xb1p, T0, T0 + L, TW), psF)
                mm(7, er(ap_pair(xb1p, T0 + 56, T0 + 56 + L, TW)), psER)
                mm(8, er(ap_pair(xb1p, T0, T0 + L, TW)), psOR)
                mm(9, er(ap_pair(xq8, b, b + 56, TW)), psER)
                mm(10, er(ap_pair(xq8, b, b + 56, TW)), psOR)

            # attention quads (window-major q/k: same slice for both branches)
            def win_ap(t_, g, w):
                return t_[32 * g:32 * g + 32, WIN * w:WIN * w + WIN]

            # quad groups of 4 windows; scores of row group g go to their
            # own psum bank: pair (g0,g1) in one 2-bank tile, (g2,g3) in the
            # next; one paired exp per 2-bank tile.
            for wq in (range(NW // 4) if go("qk") else []):
                scp = [app.tile([128, 1024], f32, name=f"scp{h}", tag="pQK",
                                bufs=2) for h in range(2)]
                p_s = [asb.tile([128, 2 * TW], f16, name=f"p_s{h}",
                                tag=f"ps{h}", bufs=3) for h in range(2)]
                for h in range(2):
                    for wi in range(4):
                        w = wq * 4 + wi
                        for g in (2 * h, 2 * h + 1):
                            sc = scp[h][:, 512 * (g % 2):512 * (g % 2) + 512]
                            nc.tensor.matmul(
                                sc[0:WIN, wi * WIN:(wi + 1) * WIN],
                                win_ap(k_wm, g, w), win_ap(q_wm, g, w),
                                start=(wi == 0), stop=True,
                                tile_position=(32 * g, 0),
                                skip_group_check=True)
                    if not go("exp"):
                        nc.vector.tensor_copy(
                            p_s[h][0:WIN, :].rearrange("p (b c) -> p b c", b=2),
                            scp[h].rearrange("p (b c) -> p b c", b=2)
                            [0:WIN, :, 0:TW])
                    else:
                        nc.scalar.activation(
                            p_s[h][0:WIN, :].rearrange("p (b c) -> p b c", b=2),
                            scp[h].rearrange("p (b c) -> p b c", b=2)
                            [0:WIN, :, 0:TW],
                            AF.Exp,
                            scale=HD_SCALE if (h == 0 or i in (0, IPC - 1))
                            else HD_SCALE / 1024.0)

                def psg(g):
                    return p_s[g // 2][:, TW * (g % 2):TW * (g % 2) + TW]

                # 448-wide sums (4 MMs) + per-window AV (16 MMs), one divide
                ps_sum = app.tile([128, 512], f32, name="ps_sum", tag="pC",
                                  bufs=2)
                ps_av = app.tile([128, 512], f32, name="ps_av", tag="pC",
                                 bufs=2)
                for g in range(4) if go("sums") else []:
                    nc.tensor.matmul(
                        ps_sum[32 * g:32 * g + 32, 0:TW],
                        ones_t[0:WIN, 0:32], psg(g)[0:WIN, 0:TW],
                        start=True, stop=True, tile_position=(0, 32 * g),
                        skip_group_check=True)
                for wi in range(4):
                    w = wq * 4 + wi
                    for g in range(4) if go("av") else []:
                        br, hd = g // 2, g % 2
                        vc = (br * NW + w) * 64 + 32 * hd
                        nc.tensor.matmul(
                            ps_av[32 * g:32 * g + 32, wi * WIN:(wi + 1) * WIN],
                            v_wm[0:WIN, vc:vc + 32],
                            psg(g)[0:WIN, wi * WIN:(wi + 1) * WIN],
                            start=True, stop=True, tile_position=(0, 32 * g),
                            skip_group_check=True)
                if go("recip"):
                    rcp = asb.tile([128, TW], f32, name="rcp")
                    nc.vector.reciprocal_approx_fast(rcp[:], ps_sum[:, 0:TW])
                if go("quads"):
                    # normalize straight into window-major o (all 128 rows);
                    # proj un-permutes branch 0 below.
                    nc.vector.tensor_tensor(
                        o_wm[:, wq * TW:(wq + 1) * TW],
                        ps_av[:, 0:TW], rcp[:], OP.mult)

            # proj: lepe taps (fp8, proj-folded) accumulate into ps_p first,
            # then branch-0 rows re-ordered to natural by a strided moving
            # view (K=64) and branch-1 window-major == natural (K=64).
            ow_b0 = o_wm.rearrange("p (w i jj) -> p i w jj",
                                   w=NW, jj=2)
            for t in range(NTILES) if go("proj") else []:
                ps_p = app.tile([128, 512], f32, name="ps_p", tag="pC",
                                bufs=2)
                if go("lepe"):
                    lepe_tile(t, ps_p)
                nc.tensor.matmul(ps_p[:, 0:TW], wprojh[:, 0:128],
                                 ow_b0[:, 8 * t:8 * t + 8, :, :],
                                 start=not go("lepe"), stop=False,
                                 tile_position=(0, 0), skip_group_check=True)
                nc.tensor.matmul(ps_p[:, 0:TW], wprojh[:, 128:256],
                                 o_wm[:, t * TW:(t + 1) * TW],
                                 start=False, stop=True,
                                 tile_position=(0, 0), skip_group_check=True)
                nc.vector.tensor_scalar(att_f[:, t * TW:(t + 1) * TW],
                                        ps_p[:, 0:TW], ATT_SCALE,
                                        projb[:, 0:1], OP.mult, OP.add)

            # transpose attened to token-major (one xbar DMA), add residual
            if go("proj"):
                att_t = ap.tile([128, IMGCOLS], f16, name="att_t", tag="att_t",
                                bufs=3)
                nc.vector.memset(att_f[:, L:IMGCOLS], 0.0)
                rchunks = (((0, 7), (7, 13), (13, 19), (19, NBI))
                           if i == IPC - 1 else ((0, 13), (13, NBI)))
                for b0, b1 in rchunks:
                    nc.sync.dma_start_transpose(
                        att_t[:, b0 * 128:b1 * 128].rearrange(
                            "p (b f) -> p b f", b=b1 - b0),
                        att_f[:, b0 * 128:b1 * 128])
                    nc.vector.tensor_tensor(
                        r1_tm[:, i * IMGCOLS + b0 * 128:
                              i * IMGCOLS + b1 * 128],
                        att_t[:, b0 * 128:b1 * 128],
                        x_tm[:, i * IMGCOLS + b0 * 128:
                             i * IMGCOLS + b1 * 128],
                        OP.add)

            if stop_after in ("qkv", "v", "lepe", "qk", "exp", "sums",
                              "av", "recip", "quads") and i == 0:
                dbg = {"qkv": q_wm, "v": v_wm}.get(
                    stop_after, o_wm)
                dump(dbg, 3072)

            if i + 2 < IPC:
                stage_in(i + 2)

            # ---- per-image LN2 (overlaps the next image's attention) ----
            if go("ln2"):
                xh2 = mfm.tile([128, IMGCOLS], f16, name="xh2_fm", tag="xh2",
                               bufs=2)
                slot = i % 2
                layernorm(r1_tm, xh_tm, f"ln2_{i}", blk0=NBI * i,
                          dst_blk0=NBI * slot,
                          chunks=(((0, 7), (7, 13), (13, 19), (19, NBI))
                                  if i == IPC - 1 else ((0, 13), (13, NBI))),
                          post_chunk=(lambda b0, b1: transpose_chunk(
                              xh2, xh_tm, slot, b0, b1)) if go("t2") else None,
                          split_merge=(i == IPC - 1))
                xh2_tiles[i] = xh2
                if go("mlp"):
                    mlp_states[i] = mlp_start(i)
            if i >= 1 and go("mlp"):
                do_mlp_tail(mlp_states.pop(i - 1))
        if go("mlp"):
            do_mlp_tail(mlp_states.pop(IPC - 1))

    if stop_after == "load":
        dump(x_tm, NT)
    if stop_after == "ln1":
        dump(xh_tm, NT)
    if stop_after == "proj":
        dump(r1_tm, NT)

    gpool.release()
    wpool.release()


def kernel(**inputs):
    from concourse.bass_utils import run_bass_kernel_spmd

    if 'nc' not in _CACHE:
        _CACHE['nc'] = build_program()
    nc = _CACHE['nc']

    w = _prep_weights(inputs)
    x = np.asarray(inputs['x'], dtype=np.float32)
    in_maps = []
    for c in range(NCORES):
        m = {'xin': np.ascontiguousarray(
            x[c * IPC:(c + 1) * IPC].reshape(NT, DIM))}
        m.update(w)
        in_maps.append(m)
    res = run_bass_kernel_spmd(nc, in_maps, list(range(NCORES)))
    out = np.empty((B, L, DIM), np.float32)
    for c in range(NCORES):
        out[c * IPC:(c + 1) * IPC] = res.results[c]['out'].reshape(IPC, L, DIM)
    return out


# revision 86
# speedup vs baseline: 1.0852x; 1.0852x over previous
"""CSWin transformer block on 8 Trainium2 NeuronCores.

Data-parallel over batch: 32 images -> 4 images per core. Inside each core a
single fused Bass/Tile program runs, per image: LN1 -> qkv -> cross-shaped
window attention (+LePE depthwise conv) -> proj -> residual -> LN2 -> MLP ->
residual, with the MLP of image i interleaved into image i+1's attention.

Layout strategy:
  - residual stream is token-major fp16 ([128 token partitions, blocks*128
    feature cols]), per-image padded to 25 blocks of 128 tokens.
  - matmul-side activations are feature-major fp16 per image [128 ch, 3136].
  - PE transposes (fp16, via identity) bridge the two.
  - attention works on per-window quads: 4 (branch, head) groups row/col
    packed onto the PE array; softmax normalization deferred to after AV
    using 448-wide column sums (ones matmul) and a single fused divide.
  - LePE = 9 shifted matmuls against host-folded (Wv * tap-weight) matrices,
    accumulated in PSUM in the same window-major layout as the attention
    output; both are pushed through proj as K=64 partial matmuls.
"""

import numpy as np

B, RESO, DIM = 32, 56, 128
L = RESO * RESO            # 3136
NCORES = 8
IPC = B // NCORES          # images per core = 4
NT = IPC * L               # tokens per core = 12544
NBI = 25                   # padded 128-token blocks per image (24.5 -> 25)
IMGCOLS = NBI * 128        # 3200 token-major cols per image
WIN = 112                  # window size (56*2)
NW = 28                    # windows per branch per image
TW = 448                   # feature-major token tile (8 image rows)
NTILES = L // TW           # 7
HD_SCALE = float(32) ** -0.5

_CACHE = {}


def _prep_weights(inputs):
    f32 = np.float32
    f16 = np.float16
    g1 = np.asarray(inputs['norm1_g'], f32)
    b1 = np.asarray(inputs['norm1_b'], f32)
    qkv_w = np.asarray(inputs['qkv_w'], f32)
    wqkv = qkv_w * g1[:, None]
    bqkv = np.asarray(inputs['qkv_b'], f32) + b1 @ qkv_w
    assert np.max(np.abs(bqkv)) == 0.0, "nonzero qkv bias path not emitted"

    # lepe tap-folded v-projections: block (br, tap) -> wqkv_v[:,64br:+64]*w[tap,c]
    wv = wqkv[:, 256:384]
    lepe_w = [np.asarray(inputs['lepe_w0'], f32), np.asarray(inputs['lepe_w1'], f32)]
    wvtap = np.zeros((128, 18 * 64), f32)
    for br in range(2):
        for ki in range(3):
            for kj in range(3):
                tap = ki * 3 + kj
                wvtap[:, (br * 9 + tap) * 64:(br * 9 + tap + 1) * 64] = (
                    wv[:, br * 64:br * 64 + 64] * lepe_w[br][ki, kj, 0, :][None, :])

    wproj = np.asarray(inputs['proj_w'], f32)
    lepe_b = np.concatenate([np.asarray(inputs['lepe_b0'], f32),
                             np.asarray(inputs['lepe_b1'], f32)])
    projb = np.asarray(inputs['proj_b'], f32) + lepe_b @ wproj

    g2 = np.asarray(inputs['norm2_g'], f32)
    b2 = np.asarray(inputs['norm2_b'], f32)
    fc1_w = np.asarray(inputs['fc1_w'], f32)
    wfc1 = fc1_w * g2[:, None]
    fc1b = np.asarray(inputs['fc1_b'], f32) + b2 @ fc1_w
    wfc2 = np.asarray(inputs['fc2_w'], f32)
    fc2b = np.asarray(inputs['fc2_b'], f32)

    # fp8 DoubleRow lepe weights folded through proj, x S8 scale (undone on
    # the att_f evac). Each MM slot is a [128(xhat ch), 2(pair), 128(proj
    # out)] stationary; pair element B may be zero (padding pair).
    import ml_dtypes
    S8 = 4096.0

    def wt(br, ki, kj):
        t = ki * 3 + kj
        return (wvtap[:, (br * 9 + t) * 64:(br * 9 + t + 1) * 64]
                @ wproj[64 * br:64 * br + 64, :]) * S8

    zz = np.zeros((128, 128), f32)
    slots = [
        (wt(0, 0, 1), wt(0, 2, 1)),   # 1  b0 row-pair, dj=0, full
        (wt(0, 0, 2), wt(0, 2, 2)),   # 2  b0 row-pair, dj=+1, even cols
        (wt(0, 0, 0), wt(0, 2, 0)),   # 3  b0 row-pair, dj=-1, odd cols
        (wt(0, 1, 1), zz),            # 4  b0 center, full
        (wt(0, 1, 2), zz),            # 5  b0 kj=+1, even cols
        (wt(0, 1, 0), zz),            # 6  b0 kj=-1, odd cols
        (wt(1, 1, 0), wt(1, 1, 2)),   # 7  b1 dj-pair, dii=0, full
        (wt(1, 2, 0), wt(1, 2, 2)),   # 8  b1 diag, dii=+1, even rows
        (wt(1, 0, 0), wt(1, 0, 2)),   # 9  b1 diag, dii=-1, odd rows
        (wt(1, 1, 1), wt(1, 2, 1)),   # 10 b1 (center,ki=+1), even rows
        (wt(1, 0, 1), wt(1, 1, 1)),   # 11 b1 (ki=-1,center), odd rows
    ]
    wlep8 = np.concatenate([np.concatenate(s, 1) for s in slots], 1)
    assert np.abs(wlep8).max() < 400.0, np.abs(wlep8).max()
    wprojh = np.concatenate([
        np.concatenate([wproj[0:64], np.zeros((64, 128), f32)], 0),
        np.concatenate([np.zeros((64, 128), f32), wproj[64:128]], 0)],
        1) * S8
    assert np.abs(wprojh).max() < 6e4

    # fp8 DoubleRow branch-1 qkv / v / fc1 weights: K=128 split into
    # channel pairs (c, c+64) matching the xh8p paired-activation layout.
    # x32 scale keeps fp8 values out of the subnormal range; undone at the
    # exp (q*k -> /1024), the v evac (/32) and the gelu (/32).
    SQ = 32.0

    def pair_k(w):  # [128, M] -> [64, 2, M] -> [64, 2*M]
        return np.concatenate([w[0:64], w[64:128]], 1)

    wqk8 = np.concatenate([
        np.concatenate([np.zeros((64, 64), f32), wqkv[0:64, 64:128] * SQ,
                        np.zeros((64, 64), f32), wqkv[64:128, 64:128] * SQ],
                       1),
        np.concatenate([np.zeros((64, 64), f32), wqkv[0:64, 192:256] * SQ,
                        np.zeros((64, 64), f32), wqkv[64:128, 192:256] * SQ],
                       1)], 1)  # [64, 512]: q-b1 pad128-pair, k-b1 pad128-pair
    wv8 = pair_k(wqkv[:, 320:384] * SQ)       # [64, 128]
    wfc18 = np.concatenate(                   # [64, 1024], per-chunk pairs
        [pair_k(wfc1[:, m * 128:(m + 1) * 128] * SQ) for m in range(4)], 1)
    for nm, arr in (("wqk8", wqk8), ("wv8", wv8), ("wfc18", wfc18)):
        assert np.abs(arr).max() < 400.0, (nm, np.abs(arr).max())

    return {
        'wqkv': wqkv.astype(f16),
        'wqk8': np.ascontiguousarray(wqk8).astype(ml_dtypes.float8_e4m3),
        'wv8': np.ascontiguousarray(wv8).astype(ml_dtypes.float8_e4m3),
        'wfc18': np.ascontiguousarray(wfc18).astype(ml_dtypes.float8_e4m3),
        'wlep8': np.ascontiguousarray(wlep8).astype(ml_dtypes.float8_e4m3),
        'wprojh': np.ascontiguousarray(wprojh).astype(f16),
        'projb': np.ascontiguousarray(projb.reshape(128, 1)),
        'wfc1': wfc1.astype(f16),
        'fc1b': np.ascontiguousarray(fc1b.reshape(4, 128).T),
        'wfc2': np.ascontiguousarray(
            (wfc2.reshape(4, 128, 128).transpose(1, 0, 2).reshape(128, 512))
            * 16.0).astype(__import__('ml_dtypes').float8_e4m3),
        'fc2b': np.ascontiguousarray(fc2b.reshape(128, 1)),
    }


def build_program(stop_after=None):
    import concourse.tile as tile
    from concourse import bacc, mybir

    dt = mybir.dt

    nc = bacc.Bacc("TRN2", target_bir_lowering=False, debug=False,
                   num_devices=NCORES)

    xin = nc.dram_tensor("xin", [NT, DIM], dt.float32, kind="ExternalInput").ap()
    wd = {}
    for name, shape, d in [
            ("wqkv", [128, 384], dt.float16),
            ("wqk8", [64, 512], dt.float8e4),
            ("wv8", [64, 128], dt.float8e4),
            ("wfc18", [64, 1024], dt.float8e4),
            ("wlep8", [128, 2816], dt.float8e4),
            ("wprojh", [128, 256], dt.float16), ("projb", [128, 1], dt.float32),
            ("wfc1", [128, 512], dt.float16), ("fc1b", [128, 4], dt.float32),
            ("wfc2", [128, 512], dt.float8e4),
            ("fc2b", [128, 1], dt.float32)]:
        wd[name] = nc.dram_tensor(name, shape, d, kind="ExternalInput").ap()
    out_d = nc.dram_tensor("out", [NT, DIM], dt.float32, kind="ExternalOutput").ap()

    with tile.TileContext(nc) as tc:
        _body(tc, mybir, xin, out_d, wd, stop_after=stop_after)
    nc.compile()
    return nc


def _body(tc, mybir, xin, out_d, wd, stop_after=None):
    nc = tc.nc
    dt = mybir.dt
    AF = mybir.ActivationFunctionType
    OP = mybir.AluOpType
    f16, f32 = dt.float16, dt.float32

    dumped = []

    def dump(t, ncols):
        """debug: DMA first min(ncols,12544) cols of [128,*] tile to out."""
        n = min(ncols, NT) // 128 * 128
        dv = out_d[0:n, :].rearrange("(b p) f -> p b f", p=128)
        nc.gpsimd.dma_start(dv, t[:, 0:n].rearrange("p (b f) -> p b f",
                                                    f=128))
        dumped.append(True)

    STAGES = ["load", "ln1", "t1", "qkv", "v", "lepe", "qk", "exp", "sums",
              "av", "recip", "quads", "proj", "ln2", "t2", "mlp"]
    lim = STAGES.index(stop_after) if stop_after else len(STAGES)

    def go(stage):
        return STAGES.index(stage) <= lim

    # ---------------- persistent pools ----------------
    wpool = tc.alloc_tile_pool(name="weights", bufs=1)
    gpool = tc.alloc_tile_pool(name="globals", bufs=1)

    wqkv = wpool.tile([128, 384], f16)
    wlep8 = wpool.tile([128, 2816], dt.float8e4)
    wprojh = wpool.tile([128, 256], f16)
    projb = wpool.tile([128, 1], f32)
    wfc1 = wpool.tile([128, 512], f16)
    fc1b = wpool.tile([128, 4], f32)
    wfc2 = wpool.tile([128, 512], dt.float8e4)
    fc2b = wpool.tile([128, 1], f32)
    ones_t = wpool.tile([128, 32], f16)
    epsb = wpool.tile([128, 1], f32)
    nc.vector.memset(epsb[:], 1e-5)
    # weight DMAs sequenced after the first image-load chunk on the (single)
    # DMA resource: they are not needed until the first qkv matmul
    with tc.tile_wait_until(0.005):
        for name, t in [("wqkv", wqkv), ("wlep8", wlep8),
                        ("wprojh", wprojh),
                        ("projb", projb), ("wfc1", wfc1), ("fc1b", fc1b),
                        ("wfc2", wfc2), ("fc2b", fc2b)]:
            nc.sync.dma_start(t[:], wd[name])
    nc.vector.memset(ones_t[:], 1.0)

    # token-major global tensors, per-image padded to 25 blocks.
    # xh_tm is a 2-image ring: each slice is written by an LN and read by
    # the immediately following transpose.
    x_tm = gpool.tile([128, IPC * IMGCOLS], f16)
    xh_tm = gpool.tile([128, 2 * IMGCOLS], f16)
    r1_tm = x_tm  # residual accumulates in place

    # fp8 natural-order xhat with 56-col zero pads top/bottom (+extra tail
    # pad so zero-weight padding pairs stay in bounds); 2-image ring,
    # per-image data DMA, pads memset once.
    xq8r = gpool.tile([128, 2 * 3360], dt.float8e4)
    for s in range(2):
        nc.gpsimd.memset(xq8r[:, s * 3360:s * 3360 + 56], 0.0)
        nc.gpsimd.memset(xq8r[:, s * 3360 + 3192:s * 3360 + 3360], 0.0)

    import bass_rust as _br

    def ap_pair(t_ap, off1, off2, n):
        """[128, 2, n] overlapping view: pair elements at cols off1/off2."""
        v = t_ap[:, off1:off1 + n].unsqueeze(1)
        raw = [list(x) for x in v.ap]
        raw[1] = [off2 - off1, 2]
        c = v.copy()
        c.ap = _br.VecI64Pair(raw)
        return c

    ATT_SCALE = 1.0 / 4096.0

    NBLK = IPC * NBI  # 100 token blocks

    # ---------------- per-image load (cast fp32->fp16), chunked ----------
    def load_img_chunk(i, b0, b1):
        src = xin[i * L:(i + 1) * L, :]
        bf = min(b1, 24)
        if bf > b0:
            full = src[b0 * 128:bf * 128, :].rearrange("(b p) f -> p b f", p=128)
            dst = x_tm[:, i * IMGCOLS + b0 * 128:i * IMGCOLS + bf * 128
                       ].rearrange("p (b f) -> p b f", b=bf - b0)
            nc.gpsimd.dma_start(dst, full)
        if b1 == NBI:
            nc.gpsimd.dma_start(
                x_tm[0:64, i * IMGCOLS + 24 * 128:i * IMGCOLS + 25 * 128],
                src[24 * 128:L, :])
            nc.gpsimd.memset(
                x_tm[64:128, i * IMGCOLS + 24 * 128:i * IMGCOLS + 25 * 128], 0.0)

    def layernorm(src_tm, dst_tm, sname, blk0, dst_blk0, chunks,
                  pre_chunk=None, post_chunk=None, newton2=False,
                  split_merge=False, apply_pool=False):
        """token-major LN over NBI blocks at blk0, processed in chunks:
        bn_stats per chunk (after pre_chunk), ONE even/odd merge + rsqrt
        (quadratic seed + Newton on DVE), then fused apply per chunk
        followed by post_chunk (e.g. a transpose). split_merge runs the
        whole pipeline per chunk (shorter latency, more DVE ops)."""
        if split_merge:
            for b0, b1 in chunks:
                layernorm(src_tm, dst_tm, f"{sname}_{b0}", blk0, dst_blk0,
                          ((b0, b1),), pre_chunk, post_chunk, newton2,
                          apply_pool=apply_pool)
            return
        nb = NBI
        with tc.tile_pool(name=f"ln_{sname}", bufs=1) as lp:
            st6 = lp.tile([128, nb * 8], f32, name=f"st6_{sname}")
            rs_t = lp.tile([128, nb], f32, name=f"rs_{sname}")
            c_t = lp.tile([128, nb], f32, name=f"c_{sname}")
            mu_t = lp.tile([128, nb], f32, name=f"mu_{sname}")
            m2_t = lp.tile([128, nb], f32, name=f"m2_{sname}")
            for b0, b1 in chunks:
                if pre_chunk is not None:
                    pre_chunk(b0, b1)
                for b in range(b0, b1):
                    nc.vector.bn_stats(st6[:, b * 8:b * 8 + 6],
                                       src_tm[:, (blk0 + b) * 128:
                                              (blk0 + b + 1) * 128])
            c0, c1 = chunks[0][0], chunks[-1][1]
            sp = slice(c0, c1)
            sv = st6.rearrange("p (b s) -> p b s", s=8)
            # stats6 = [cnt_e, mean_e, cnt*var_e, cnt_o, mean_o, cnt*var_o]
            nc.vector.tensor_tensor(mu_t[:, sp], sv[:, sp, 1:2], sv[:, sp, 4:5], OP.add)
            nc.vector.tensor_tensor(c_t[:, sp], sv[:, sp, 1:2], sv[:, sp, 4:5], OP.subtract)
            nc.vector.tensor_tensor(c_t[:, sp], c_t[:, sp], c_t[:, sp], OP.mult)
            nc.vector.tensor_tensor(m2_t[:, sp], sv[:, sp, 2:3], sv[:, sp, 5:6], OP.add)
            nc.vector.scalar_tensor_tensor(m2_t[:, sp], c_t[:, sp], 32.0, m2_t[:, sp],
                                           OP.mult, OP.add)
            t_t = lp.tile([128, nb], f32, name=f"t_{sname}")
            nc.vector.tensor_scalar(m2_t[:, sp], m2_t[:, sp], 1.0 / 128.0, 1e-5,
                                    OP.mult, OP.add)
            nc.vector.tensor_scalar(rs_t[:, sp], m2_t[:, sp], 0.23645927242441878,
                                    -1.0257861053814088, OP.mult, OP.add)
            nc.vector.tensor_tensor(rs_t[:, sp], rs_t[:, sp], m2_t[:, sp], OP.mult)
            nc.vector.tensor_scalar(rs_t[:, sp], rs_t[:, sp], 1.8125565144482214,
                                    None, OP.add)
            for _ in range(2 if newton2 else 1):
                nc.vector.tensor_tensor(t_t[:, sp], rs_t[:, sp], rs_t[:, sp], OP.mult)
                nc.vector.tensor_tensor(t_t[:, sp], t_t[:, sp], m2_t[:, sp], OP.mult)
                nc.vector.tensor_scalar(t_t[:, sp], t_t[:, sp], -0.5, 1.5,
                                        OP.mult, OP.add)
                nc.vector.tensor_tensor(rs_t[:, sp], rs_t[:, sp], t_t[:, sp], OP.mult)
            # c = -(mu_sum/2)*rs
            nc.vector.scalar_tensor_tensor(c_t[:, sp], mu_t[:, sp], -0.5, rs_t[:, sp],
                                           OP.mult, OP.mult)
            aeng = nc.gpsimd if apply_pool else nc.vector
            for b0, b1 in chunks:
                for b in range(b0, b1):
                    aeng.tensor_scalar(
                        dst_tm[:, (dst_blk0 + b) * 128:(dst_blk0 + b + 1) * 128],
                        src_tm[:, (blk0 + b) * 128:(blk0 + b + 1) * 128],
                        rs_t[:, b:b + 1], c_t[:, b:b + 1], OP.mult, OP.add)
                if post_chunk is not None:
                    post_chunk(b0, b1)

    def transpose_chunk(dst, src_tm, slot, b0, b1):
        """xbar DMA transpose of token-major blocks [b0:b1] of ring slot into
        feature-major dst cols [b0*128:b1*128)."""
        nc.sync.dma_start_transpose(
            dst[:, b0 * 128:b1 * 128].rearrange("p (b f) -> p b f", b=b1 - b0),
            src_tm[:, slot * IMGCOLS + b0 * 128:slot * IMGCOLS + b1 * 128])

    # ---------------- fused per-image loop ----------------
    with tc.tile_pool(name="att_fm", bufs=1) as ap, \
         tc.tile_pool(name="att_ps", bufs=1, space="PSUM") as app, \
         tc.tile_pool(name="att_sb", bufs=3) as asb, \
         tc.tile_pool(name="mlp_fm", bufs=1) as mfm, \
         tc.tile_pool(name="mlp_sb", bufs=2) as msb:
        xh_tiles = {}
        xh2_tiles = {}
        mlp_states = {}

        def epi_chunk(i, mlp_f, b0, b1):
            """transpose + residual + store of output blocks [b0:b1).
            Non-tail images add in fp16 and let the (gpsimd) store DMA
            upcast — halves the DVE cost of the residual adds; the last
            image keeps fp32 + HWDGE stores for the shortest drain."""
            nb = b1 - b0
            fin = True
            mlp_t = mfm.tile([128, 1024], f16, name="mlp_t", tag="mlp_t",
                             bufs=2)
            o_sb = mfm.tile([128, 1024], f32, name="o_sb",
                            tag="o_sbF", bufs=2)
            nc.sync.dma_start_transpose(
                mlp_t[:, 0:nb * 128].rearrange("p (b f) -> p b f", b=nb),
                mlp_f[:, b0 * 128:b1 * 128])
            nc.vector.tensor_tensor(
                o_sb[:, 0:nb * 128], mlp_t[:, 0:nb * 128],
                r1_tm[:, i * IMGCOLS + b0 * 128:i * IMGCOLS + b1 * 128],
                OP.add)
            eng = nc.sync if fin else nc.gpsimd
            nfull = min(b1, 24) - b0
            if nfull > 0:
                dst = out_d[i * L + b0 * 128:i * L + (b0 + nfull) * 128, :
                            ].rearrange("(b p) f -> p b f", p=128)
                eng.dma_start(dst, o_sb[:, 0:nfull * 128].rearrange(
                    "p (b f) -> p b f", b=nfull))
            if b1 == 25:
                # tail: blocks 24..24.5 (64 tokens)
                eng.dma_start(out_d[i * L + 24 * 128:(i + 1) * L, :],
                              o_sb[0:64, nfull * 128:nfull * 128 + 128])

        def mlp_start(i):
            xh2 = xh2_tiles.pop(i)
            xh28p = None
            return {
                'i': i,
                'xh2': xh2,
                'xh28p': xh28p,
                'mlp_f': mfm.tile([128, IMGCOLS], f16, name="mlp_f",
                                  tag="mlp_f", bufs=2),
                'h2': {},
            }

        def mlp_fc1(st, t):
            rhs = st['xh2'][:, t * TW:(t + 1) * TW]
            rhs8 = None
            if st['xh28p'] is not None:
                rhs8 = st['xh28p'].rearrange("p (two c) -> p two c", two=2)[
                    :, :, t * TW:(t + 1) * TW]
            h2s = []
            for half in range(2):
                h2 = msb.tile([128, 2 * TW], dt.float8e4,
                              name=f"h2_{half}", tag="hs", bufs=8)
                ps_h = app.tile([128, 1024], f32, name=f"ps_h{half}",
                                tag="pQK", bufs=2)
                for mm in range(2):
                    m = 2 * half + mm
                    if rhs8 is not None:
                        nc.tensor.matmul(
                            ps_h[:, mm * 512:mm * 512 + TW],
                            wfc18[:, m * 256:(m + 1) * 256].rearrange(
                                "p (two c) -> p two c", two=2),
                            rhs8, start=True, stop=True,
                            perf_mode=mybir.MatmulPerfMode.DoubleRow,
                            skip_group_check=True)
                    else:
                        nc.tensor.matmul(ps_h[:, mm * 512:mm * 512 + TW],
                                         wfc1[:, m * 128:(m + 1) * 128],
                                         rhs, start=True, stop=True,
                                         skip_group_check=True)
                nc.scalar.activation(
                    h2.rearrange("p (m c) -> p m c", m=2),
                    ps_h.rearrange("p (m c) -> p m c", m=2)[:, :, 0:TW],
                    AF.Gelu,
                    scale=(1.0 / 32.0) if rhs8 is not None else 1.0)
                h2s.append(h2)
            st['h2'][t] = h2s

        def mlp_fc2(st, t):
            i = st['i']
            mlp_f = st['mlp_f']
            h2s = st['h2'].pop(t)
            ps_o = app.tile([128, 512], f32, name="ps_o", tag="pC", bufs=2)
            # fc2: fp8 DoubleRow pairs two K=128 chunks per pass (weights
            # host-scaled x16; undone in the bias add below)
            for half in range(2):
                nc.tensor.matmul(
                    ps_o[:, 0:TW],
                    wfc2[:, half * 256:(half + 1) * 256].rearrange(
                        "p (two m) -> p two m", two=2),
                    h2s[half].rearrange("p (two c) -> p two c", two=2),
                    start=(half == 0), stop=(half == 1),
                    perf_mode=mybir.MatmulPerfMode.DoubleRow,
                    skip_group_check=True)
            nc.vector.tensor_scalar(mlp_f[:, t * TW:(t + 1) * TW],
                                    ps_o[:, 0:TW], 1.0 / 16.0,
                                    fc2b[:, 0:1], OP.mult, OP.add)
            # stream the epilogue as its input columns land
            if t == 2:
                epi_chunk(i, mlp_f, 0, 8)
            elif t == 4:
                epi_chunk(i, mlp_f, 8, 16)
            elif t == 5:
                epi_chunk(i, mlp_f, 16, 20)
            elif t == 6:
                epi_chunk(i, mlp_f, 20, 24)
                nc.vector.memset(mlp_f[:, L:IMGCOLS], 0.0)
                epi_chunk(i, mlp_f, 24, 25)

        def do_mlp_tail(st):
            if st['xh28p'] is not None:
                xh2 = st['xh2']
                nc.gpsimd.dma_start(st['xh28p'][:, 0:IMGCOLS], xh2[0:64, :])
                nc.gpsimd.dma_start(st['xh28p'][:, IMGCOLS:2 * IMGCOLS],
                                    xh2[64:128, :])
            for t in range(NTILES):
                mlp_fc1(st, t)
                if t >= 1:
                    mlp_fc2(st, t - 1)
            mlp_fc2(st, NTILES - 1)

        def stage_in(i):
            xh = ap.tile([128, IMGCOLS], f16, name="xh_fm", tag="xh", bufs=2)
            slot = i % 2

            def load(b0, b1):
                # keep prefetched images' loads well out of the startup and
                # per-image copy windows: they pollute the rotating DMA
                # semaphores and hog the DMA engines otherwise
                with tc.tile_wait_until(0.05 * (i - 1), enable=i >= 2):
                    load_img_chunk(i, b0, b1)

            layernorm(x_tm, xh_tm, f"ln1_{i}", blk0=NBI * i,
                      dst_blk0=NBI * slot, chunks=((0, 13), (13, NBI)),
                      pre_chunk=load,
                      post_chunk=lambda b0, b1: transpose_chunk(
                          xh, xh_tm, slot, b0, b1),
                      split_merge=(i < 2))
            xh_tiles[i] = xh

        stage_in(0)
        stage_in(1)
        for i in range(IPC):
            xh = xh_tiles[i]
            # fp8 copies of xhat for the DoubleRow lepe taps, issued first:
            # natural-order padded xq8 + the (j-1, j+1) edge-zeroed shifted
            # pair xb1p. (Consumed only at the lepe+proj stage — a full
            # image of slack.)
            xq8 = xq8r[:, (i % 2) * 3360:(i % 2) * 3360 + 3360]
            xb1p = ap.tile([128, 2 * L], dt.float8e4, name="xb1p", bufs=2)
            fp16_img = True
            nc.gpsimd.dma_start(xq8[:, 56:3192], xh[:, 0:L])
            nc.gpsimd.memset(xb1p[:, 0:1], 0.0)
            nc.gpsimd.memset(xb1p[:, 2 * L - 1:2 * L], 0.0)
            nc.gpsimd.dma_start(xb1p[:, 1:L], xh[:, 0:L - 1])
            nc.gpsimd.dma_start(xb1p[:, L:2 * L - 1], xh[:, 1:L])
            xb1e = xb1p.rearrange("p (c i j) -> p c i j", c=2, j=RESO)
            nc.gpsimd.memset(xb1e[:, 0, :, 0:1], 0.0)
            nc.gpsimd.memset(xb1e[:, 1, :, 55:56], 0.0)
            # q/k storage: rows 0:64 (branch0) in window-major cols
            # (112w + 2i + jj), rows 64:128 (branch1) natural (= window-major)
            qk_wm = ap.tile([128, 2 * L], f16, name="qk_wm", bufs=2)
            q_wm = qk_wm[:, 0:L]
            k_wm = qk_wm[:, L:2 * L]
            xh_b0wm = ap.tile([128, L], f16, name="xh_b0wm", bufs=1)
            v_wm = ap.tile([128, 2 * NW * 64], f16, name="v_wm", bufs=1)
            o_wm = ap.tile([128, L], f16, name="o_wm")
            att_f = ap.tile([128, IMGCOLS], f16, name="att_f")

            # qkv: q,k. branch0 halves computed directly in window-major
            # order by a window-ordered moving-operand gather; branch1
            # window-major == natural. One paired evac per tile.
            xq0 = xh[:, 0:L].rearrange("p (i w jj) -> p w i jj", w=NW, jj=2)
            qkv = qk_wm.rearrange("p (qk c) -> p qk c", qk=2)
            for t in range(NTILES) if go("qkv") else []:
                ps_qk = app.tile([128, 1024], f32, name="ps_qk", tag="pQ2",
                                 bufs=1)
                ps_q = ps_qk[:, 0:512]
                ps_k = ps_qk[:, 512:1024]
                DRm = mybir.MatmulPerfMode.DoubleRow
                rhs_wm = xq0[:, 4 * t:4 * t + 4, :, :]
                if fp16_img:
                    # startup/tail image: fp16 branch1 (no xh8p dependency)
                    rhs_nat = xh[:, t * TW:(t + 1) * TW]
                    nc.tensor.matmul(ps_q[64:128, 0:TW], wqkv[:, 64:128],
                                     rhs_nat, start=True, stop=True,
                                     tile_position=(0, 64),
                                     skip_group_check=True)
                    nc.tensor.matmul(ps_q[0:64, 0:TW], wqkv[:, 0:64], rhs_wm,
                                     start=True, stop=True,
                                     tile_position=(0, 0),
                                     skip_group_check=True)
                    nc.tensor.matmul(ps_k[64:128, 0:TW], wqkv[:, 192:256],
                                     rhs_nat, start=True, stop=True,
                                     tile_position=(0, 64),
                                     skip_group_check=True)
                    nc.tensor.matmul(ps_k[0:64, 0:TW], wqkv[:, 128:192],
                                     rhs_wm, start=True, stop=True,
                                     tile_position=(0, 0),
                                     skip_group_check=True)
                else:
                    rhs_nat8 = xh8p.rearrange("p (two c) -> p two c", two=2)[
                        :, :, t * TW:(t + 1) * TW]
                    # branch1 halves via fp8 DoubleRow (zero-padded to dst
                    # base 0, writes all 128 rows), then branch0 fp16 accums
                    nc.tensor.matmul(ps_q[:, 0:TW],
                                     wqk8[:, 0:256].rearrange(
                                         "p (two m) -> p two m", two=2),
                                     rhs_nat8, start=True, stop=False,
                                     perf_mode=DRm, tile_position=(0, 0),
                                     skip_group_check=True)
                    nc.tensor.matmul(ps_q[0:64, 0:TW], wqkv[:, 0:64], rhs_wm,
                                     start=False, stop=True,
                                     tile_position=(0, 0),
                                     skip_group_check=True)
                    nc.tensor.matmul(ps_k[:, 0:TW],
                                     wqk8[:, 256:512].rearrange(
                                         "p (two m) -> p two m", two=2),
                                     rhs_nat8, start=True, stop=False,
                                     perf_mode=DRm, tile_position=(0, 0),
                                     skip_group_check=True)
                    nc.tensor.matmul(ps_k[0:64, 0:TW], wqkv[:, 128:192],
                                     rhs_wm, start=False, stop=True,
                                     tile_position=(0, 0),
                                     skip_group_check=True)
                # split evac: k on DVE, q on Act — the single psum buffer
                # frees as fast as the next tile's matmuls need it
                nc.vector.tensor_copy(k_wm[:, t * TW:(t + 1) * TW],
                                      ps_qk[:, 512:512 + TW])
                nc.scalar.copy(q_wm[:, t * TW:(t + 1) * TW],
                               ps_qk[:, 0:TW])

            # window-major xhat copy for branch-0 stationary operands.
            # The last image's copy runs on DVE: Pool's queue is deep with
            # LN applies and fp8 copies right then, and v-b0 would stall.
            geng = nc.gpsimd
            xb0 = xh_b0wm.rearrange("p (w q2 jj) -> p q2 w jj", w=NW, q2=RESO)
            for t in range(NTILES) if go("v") else []:
                geng.tensor_copy(xb0[:, 8 * t:8 * t + 8, :, :],
                                 xh[:, t * TW:(t + 1) * TW])

            # v window-major; branch1 via fp8 DoubleRow (x32 weights,
            # undone at the evac)
            for br in range(2) if go("v") else []:
                for wg in range(NW // 4):
                    ps_v = app.tile([128, 512], f32, name="ps_v", tag="pC",
                                    bufs=2)
                    for wi in range(4):
                        w = wg * 4 + wi
                        if br == 0:
                            nc.tensor.matmul(
                                ps_v[0:WIN, wi * 64:wi * 64 + 64],
                                xh_b0wm[:, WIN * w:WIN * w + WIN],
                                wqkv[:, 256:320], start=True, stop=True)
                        elif fp16_img:
                            nc.tensor.matmul(
                                ps_v[0:WIN, wi * 64:wi * 64 + 64],
                                xh[:, WIN * w:WIN * w + WIN],
                                wqkv[:, 320:384], start=True, stop=True)
                        else:
                            nc.tensor.matmul(
                                ps_v[0:WIN, wi * 64:wi * 64 + 64],
                                xh8v[:, :, WIN * w:WIN * w + WIN],
                                wv8v, start=True, stop=True,
                                perf_mode=mybir.MatmulPerfMode.DoubleRow)
                    if br == 0:
                        nc.scalar.copy(
                            v_wm[0:WIN, (wg * 4) * 64:(wg * 4 + 4) * 64],
                            ps_v[0:WIN, 0:256])
                    elif fp16_img:
                        nc.scalar.copy(
                            v_wm[0:WIN, (NW + wg * 4) * 64:(NW + wg * 4 + 4) * 64],
                            ps_v[0:WIN, 0:256])
                    else:
                        nc.scalar.activation(
                            v_wm[0:WIN, (NW + wg * 4) * 64:(NW + wg * 4 + 4) * 64],
                            ps_v[0:WIN, 0:256], AF.Copy, scale=1.0 / 32.0)

            def lepe_tile(t, ps_p):
                """9-tap depthwise LePE conv x Wproj, natural token order,
                accumulated into the proj psum: 11 fp8 DoubleRow MMs with
                parity masking via strided psum writes."""
                DR = mybir.MatmulPerfMode.DoubleRow
                T0 = TW * t
                b = 56 + T0  # xq8 col of token T0

                def wsl(k):
                    return wlep8[:, k * 256:k * 256 + 256].rearrange(
                        "p (two m) -> p two m", two=2)

                def mm(k, mov, out, start=False):
                    nc.tensor.matmul(out, wsl(k), mov, start=start,
                                     stop=False, perf_mode=DR,
                                     tile_position=(0, 0),
                                     skip_group_check=True)

                psF = ps_p[:, 0:TW]
                psc = ps_p[:, 0:TW].rearrange("p (c s) -> p c s", s=2)
                psE, psO = psc[:, :, 0:1], psc[:, :, 1:2]
                psr = ps_p[:, 0:TW].rearrange("p (r c) -> p r c", c=56)
                psER, psOR = psr[:, 0:8:2, :], psr[:, 1:8:2, :]

                def ev(pair):  # stride-2 moving view (base = first element)
                    return pair.rearrange("p two (c s) -> p two c s",
                                          s=2)[:, :, :, 0:1]

                def er(pair):  # every-other-56-block moving view
                    return pair.rearrange("p two (r c) -> p two r c",
                                          c=56)[:, :, 0:8:2, :]

                # branch 0 (vertical stripes)
                mm(0, ap_pair(xq8, b - 56, b + 56, TW), psF, start=True)
                mm(1, ev(ap_pair(xq8, b - 55, b + 57, TW)), psE)
                mm(2, ev(ap_pair(xq8, b - 56, b + 56, TW)), psO)
                mm(3, ap_pair(xq8, b, b + 112, TW), psF)
                mm(4, ev(ap_pair(xq8, b + 1, b + 113, TW)), psE)
                mm(5, ev(ap_pair(xq8, b, b + 112, TW)), psO)
                # branch 1 (horizontal stripes); xb1p halves: h0[c]=x[c-1]
                # (kj=0 tap), h1[c]=x[c+1] (kj=2), j-edge zeroed
                mm(6, ap_pair(xb1p, T0, T0 + L, TW), psF)
                mm(7, er(ap_pair(xb1p, T0 + 56, T0 + 56 + L, TW)), psER)
                mm(8, er(ap_pair(xb1p, T0, T0 + L, TW)), psOR)
                mm(9, er(ap_pair(xq8, b, b + 56, TW)), psER)
                mm(10, er(ap_pair(xq8, b, b + 56, TW)), psOR)

            # attention quads (window-major q/k: same slice for both branches)
            def win_ap(t_, g, w):
                return t_[32 * g:32 * g + 32, WIN * w:WIN * w + WIN]

            # quad groups of 4 windows; scores of row group g go to their
            # own psum bank: pair (g0,g1) in one 2-bank tile, (g2,g3) in the
            # next; one paired exp per 2-bank tile.
            for wq in (range(NW // 4) if go("qk") else []):
                scp = [app.tile([128, 1024], f32, name=f"scp{h}", tag="pQK",
                                bufs=2) for h in range(2)]
                p_s = [asb.tile([128, 2 * TW], f16, name=f"p_s{h}",
                                tag=f"ps{h}", bufs=3) for h in range(2)]
                for h in range(2):
                    for wi in range(4):
                        w = wq * 4 + wi
                        for g in (2 * h, 2 * h + 1):
                            sc = scp[h][:, 512 * (g % 2):512 * (g % 2) + 512]
                            nc.tensor.matmul(
                                sc[0:WIN, wi * WIN:(wi + 1) * WIN],
                                win_ap(k_wm, g, w), win_ap(q_wm, g, w),
                                start=(wi == 0), stop=True,
                                tile_position=(32 * g, 0),
                                skip_group_check=True)
                    if not go("exp"):
                        nc.vector.tensor_copy(
                            p_s[h][0:WIN, :].rearrange("p (b c) -> p b c", b=2),
                            scp[h].rearrange("p (b c) -> p b c", b=2)
                            [0:WIN, :, 0:TW])
                    else:
                        nc.scalar.activation(
                            p_s[h][0:WIN, :].rearrange("p (b c) -> p b c", b=2),
                            scp[h].rearrange("p (b c) -> p b c", b=2)
                            [0:WIN, :, 0:TW],
                            AF.Exp,
                            scale=HD_SCALE if (h == 0 or i in (0, IPC - 1))
                            else HD_SCALE / 1024.0)

                def psg(g):
                    return p_s[g // 2][:, TW * (g % 2):TW * (g % 2) + TW]

                # 448-wide sums (4 MMs) + per-window AV (16 MMs), one divide
                ps_sum = app.tile([128, 512], f32, name="ps_sum", tag="pC",
                                  bufs=2)
                ps_av = app.tile([128, 512], f32, name="ps_av", tag="pC",
                                 bufs=2)
                for g in range(4) if go("sums") else []:
                    nc.tensor.matmul(
                        ps_sum[32 * g:32 * g + 32, 0:TW],
                        ones_t[0:WIN, 0:32], psg(g)[0:WIN, 0:TW],
                        start=True, stop=True, tile_position=(0, 32 * g),
                        skip_group_check=True)
                for wi in range(4):
                    w = wq * 4 + wi
                    for g in range(4) if go("av") else []:
                        br, hd = g // 2, g % 2
                        vc = (br * NW + w) * 64 + 32 * hd
                        nc.tensor.matmul(
                            ps_av[32 * g:32 * g + 32, wi * WIN:(wi + 1) * WIN],
                            v_wm[0:WIN, vc:vc + 32],
                            psg(g)[0:WIN, wi * WIN:(wi + 1) * WIN],
                            start=True, stop=True, tile_position=(0, 32 * g),
                            skip_group_check=True)
                if go("recip"):
                    rcp = asb.tile([128, TW], f32, name="rcp")
                    nc.vector.reciprocal_approx_fast(rcp[:], ps_sum[:, 0:TW])
                if go("quads"):
                    # normalize straight into window-major o (all 128 rows);
                    # proj un-permutes branch 0 below.
                    nc.vector.tensor_tensor(
                        o_wm[:, wq * TW:(wq + 1) * TW],
                        ps_av[:, 0:TW], rcp[:], OP.mult)

            # proj: lepe taps (fp8, proj-folded) accumulate into ps_p first,
            # then branch-0 rows re-ordered to natural by a strided moving
            # view (K=64) and branch-1 window-major == natural (K=64).
            ow_b0 = o_wm.rearrange("p (w i jj) -> p i w jj",
                                   w=NW, jj=2)
            for t in range(NTILES) if go("proj") else []:
                ps_p = app.tile([128, 512], f32, name="ps_p", tag="pC",
                                bufs=2)
                if go("lepe"):
                    lepe_tile(t, ps_p)
                nc.tensor.matmul(ps_p[:, 0:TW], wprojh[:, 0:128],
                                 ow_b0[:, 8 * t:8 * t + 8, :, :],
                                 start=not go("lepe"), stop=False,
                                 tile_position=(0, 0), skip_group_check=True)
                nc.tensor.matmul(ps_p[:, 0:TW], wprojh[:, 128:256],
                                 o_wm[:, t * TW:(t + 1) * TW],
                                 start=False, stop=True,
                                 tile_position=(0, 0), skip_group_check=True)
                nc.vector.tensor_scalar(att_f[:, t * TW:(t + 1) * TW],
                                        ps_p[:, 0:TW], ATT_SCALE,
                                        projb[:, 0:1], OP.mult, OP.add)

            # transpose attened to token-major (one xbar DMA), add residual
            if go("proj"):
                att_t = ap.tile([128, IMGCOLS], f16, name="att_t", tag="att_t",
                                bufs=3)
                nc.vector.memset(att_f[:, L:IMGCOLS], 0.0)
                rchunks = (((0, 7), (7, 13), (13, 19), (19, NBI))
                           if i == IPC - 1 else ((0, 13), (13, NBI)))
                for b0, b1 in rchunks:
                    nc.sync.dma_start_transpose(
                        att_t[:, b0 * 128:b1 * 128].rearrange(
                            "p (b f) -> p b f", b=b1 - b0),
                        att_f[:, b0 * 128:b1 * 128])
                    nc.vector.tensor_tensor(
                        r1_tm[:, i * IMGCOLS + b0 * 128:
                              i * IMGCOLS + b1 * 128],
                        att_t[:, b0 * 128:b1 * 128],
                        x_tm[:, i * IMGCOLS + b0 * 128:
                             i * IMGCOLS + b1 * 128],
                        OP.add)

            if stop_after in ("qkv", "v", "lepe", "qk", "exp", "sums",
                              "av", "recip", "quads") and i == 0:
                dbg = {"qkv": q_wm, "v": v_wm}.get(
                    stop_after, o_wm)
                dump(dbg, 3072)

            if i + 2 < IPC:
                stage_in(i + 2)

            # ---- per-image LN2 (overlaps the next image's attention) ----
            if go("ln2"):
                xh2 = mfm.tile([128, IMGCOLS], f16, name="xh2_fm", tag="xh2",
                               bufs=2)
                slot = i % 2
                layernorm(r1_tm, xh_tm, f"ln2_{i}", blk0=NBI * i,
                          dst_blk0=NBI * slot,
                          chunks=(((0, 7), (7, 13), (13, 19), (19, NBI))
                                  if i == IPC - 1 else ((0, 13), (13, NBI))),
                          post_chunk=(lambda b0, b1: transpose_chunk(
                              xh2, xh_tm, slot, b0, b1)) if go("t2") else None,
                          split_merge=(i == IPC - 1))
                xh2_tiles[i] = xh2
                if go("mlp"):
                    mlp_states[i] = mlp_start(i)
            if i >= 1 and go("mlp"):
                do_mlp_tail(mlp_states.pop(i - 1))
        if go("mlp"):
            do_mlp_tail(mlp_states.pop(IPC - 1))

    if stop_after == "load":
        dump(x_tm, NT)
    if stop_after == "ln1":
        dump(xh_tm, NT)
    if stop_after == "proj":
        dump(r1_tm, NT)

    gpool.release()
    wpool.release()


def kernel(**inputs):
    from concourse.bass_utils import run_bass_kernel_spmd

    if 'nc' not in _CACHE:
        _CACHE['nc'] = build_program()
    nc = _CACHE['nc']

    w = _prep_weights(inputs)
    x = np.asarray(inputs['x'], dtype=np.float32)
    in_maps = []
    for c in range(NCORES):
        m = {'xin': np.ascontiguousarray(
            x[c * IPC:(c + 1) * IPC].reshape(NT, DIM))}
        m.update(w)
        in_maps.append(m)
    res = run_bass_kernel_spmd(nc, in_maps, list(range(NCORES)))
    out = np.empty((B, L, DIM), np.float32)
    for c in range(NCORES):
        out[c * IPC:(c + 1) * IPC] = res.results[c]['out'].reshape(IPC, L, DIM)
    return out


# revision 87
# speedup vs baseline: 1.0890x; 1.0035x over previous
"""CSWin transformer block on 8 Trainium2 NeuronCores.

Data-parallel over batch: 32 images -> 4 images per core. Inside each core a
single fused Bass/Tile program runs, per image: LN1 -> qkv -> cross-shaped
window attention (+LePE depthwise conv) -> proj -> residual -> LN2 -> MLP ->
residual, with the MLP of image i interleaved into image i+1's attention.

Layout strategy:
  - residual stream is token-major fp16 ([128 token partitions, blocks*128
    feature cols]), per-image padded to 25 blocks of 128 tokens.
  - matmul-side activations are feature-major fp16 per image [128 ch, 3136].
  - PE transposes (fp16, via identity) bridge the two.
  - attention works on per-window quads: 4 (branch, head) groups row/col
    packed onto the PE array; softmax normalization deferred to after AV
    using 448-wide column sums (ones matmul) and a single fused divide.
  - LePE = 9 shifted matmuls against host-folded (Wv * tap-weight) matrices,
    accumulated in PSUM in the same window-major layout as the attention
    output; both are pushed through proj as K=64 partial matmuls.
"""

import numpy as np

B, RESO, DIM = 32, 56, 128
L = RESO * RESO            # 3136
NCORES = 8
IPC = B // NCORES          # images per core = 4
NT = IPC * L               # tokens per core = 12544
NBI = 25                   # padded 128-token blocks per image (24.5 -> 25)
IMGCOLS = NBI * 128        # 3200 token-major cols per image
WIN = 112                  # window size (56*2)
NW = 28                    # windows per branch per image
TW = 448                   # feature-major token tile (8 image rows)
NTILES = L // TW           # 7
HD_SCALE = float(32) ** -0.5

_CACHE = {}


def _prep_weights(inputs):
    f32 = np.float32
    f16 = np.float16
    g1 = np.asarray(inputs['norm1_g'], f32)
    b1 = np.asarray(inputs['norm1_b'], f32)
    qkv_w = np.asarray(inputs['qkv_w'], f32)
    wqkv = qkv_w * g1[:, None]
    bqkv = np.asarray(inputs['qkv_b'], f32) + b1 @ qkv_w
    assert np.max(np.abs(bqkv)) == 0.0, "nonzero qkv bias path not emitted"

    # lepe tap-folded v-projections: block (br, tap) -> wqkv_v[:,64br:+64]*w[tap,c]
    wv = wqkv[:, 256:384]
    lepe_w = [np.asarray(inputs['lepe_w0'], f32), np.asarray(inputs['lepe_w1'], f32)]
    wvtap = np.zeros((128, 18 * 64), f32)
    for br in range(2):
        for ki in range(3):
            for kj in range(3):
                tap = ki * 3 + kj
                wvtap[:, (br * 9 + tap) * 64:(br * 9 + tap + 1) * 64] = (
                    wv[:, br * 64:br * 64 + 64] * lepe_w[br][ki, kj, 0, :][None, :])

    wproj = np.asarray(inputs['proj_w'], f32)
    lepe_b = np.concatenate([np.asarray(inputs['lepe_b0'], f32),
                             np.asarray(inputs['lepe_b1'], f32)])
    projb = np.asarray(inputs['proj_b'], f32) + lepe_b @ wproj

    g2 = np.asarray(inputs['norm2_g'], f32)
    b2 = np.asarray(inputs['norm2_b'], f32)
    fc1_w = np.asarray(inputs['fc1_w'], f32)
    wfc1 = fc1_w * g2[:, None]
    fc1b = np.asarray(inputs['fc1_b'], f32) + b2 @ fc1_w
    wfc2 = np.asarray(inputs['fc2_w'], f32)
    fc2b = np.asarray(inputs['fc2_b'], f32)

    # fp8 DoubleRow lepe weights folded through proj, x S8 scale (undone on
    # the att_f evac). Each MM slot is a [128(xhat ch), 2(pair), 128(proj
    # out)] stationary; pair element B may be zero (padding pair).
    import ml_dtypes
    S8 = 4096.0

    def wt(br, ki, kj):
        t = ki * 3 + kj
        return (wvtap[:, (br * 9 + t) * 64:(br * 9 + t + 1) * 64]
                @ wproj[64 * br:64 * br + 64, :]) * S8

    zz = np.zeros((128, 128), f32)
    slots = [
        (wt(0, 0, 1), wt(0, 2, 1)),   # 1  b0 row-pair, dj=0, full
        (wt(0, 0, 2), wt(0, 2, 2)),   # 2  b0 row-pair, dj=+1, even cols
        (wt(0, 0, 0), wt(0, 2, 0)),   # 3  b0 row-pair, dj=-1, odd cols
        (wt(0, 1, 1), zz),            # 4  b0 center, full
        (wt(0, 1, 2), zz),            # 5  b0 kj=+1, even cols
        (wt(0, 1, 0), zz),            # 6  b0 kj=-1, odd cols
        (wt(1, 1, 0), wt(1, 1, 2)),   # 7  b1 dj-pair, dii=0, full
        (wt(1, 2, 0), wt(1, 2, 2)),   # 8  b1 diag, dii=+1, even rows
        (wt(1, 0, 0), wt(1, 0, 2)),   # 9  b1 diag, dii=-1, odd rows
        (wt(1, 1, 1), wt(1, 2, 1)),   # 10 b1 (center,ki=+1), even rows
        (wt(1, 0, 1), wt(1, 1, 1)),   # 11 b1 (ki=-1,center), odd rows
    ]
    wlep8 = np.concatenate([np.concatenate(s, 1) for s in slots], 1)
    assert np.abs(wlep8).max() < 400.0, np.abs(wlep8).max()
    wprojh = np.concatenate([
        np.concatenate([wproj[0:64], np.zeros((64, 128), f32)], 0),
        np.concatenate([np.zeros((64, 128), f32), wproj[64:128]], 0)],
        1) * S8
    assert np.abs(wprojh).max() < 6e4

    # fp8 DoubleRow branch-1 qkv / v / fc1 weights: K=128 split into
    # channel pairs (c, c+64) matching the xh8p paired-activation layout.
    # x32 scale keeps fp8 values out of the subnormal range; undone at the
    # exp (q*k -> /1024), the v evac (/32) and the gelu (/32).
    SQ = 32.0

    def pair_k(w):  # [128, M] -> [64, 2, M] -> [64, 2*M]
        return np.concatenate([w[0:64], w[64:128]], 1)

    wqk8 = np.concatenate([
        np.concatenate([np.zeros((64, 64), f32), wqkv[0:64, 64:128] * SQ,
                        np.zeros((64, 64), f32), wqkv[64:128, 64:128] * SQ],
                       1),
        np.concatenate([np.zeros((64, 64), f32), wqkv[0:64, 192:256] * SQ,
                        np.zeros((64, 64), f32), wqkv[64:128, 192:256] * SQ],
                       1)], 1)  # [64, 512]: q-b1 pad128-pair, k-b1 pad128-pair
    wv8 = pair_k(wqkv[:, 320:384] * SQ)       # [64, 128]
    wfc18 = np.concatenate(                   # [64, 1024], per-chunk pairs
        [pair_k(wfc1[:, m * 128:(m + 1) * 128] * SQ) for m in range(4)], 1)
    for nm, arr in (("wqk8", wqk8), ("wv8", wv8), ("wfc18", wfc18)):
        assert np.abs(arr).max() < 400.0, (nm, np.abs(arr).max())

    return {
        'wqkv': wqkv.astype(f16),
        'wqk8': np.ascontiguousarray(wqk8).astype(ml_dtypes.float8_e4m3),
        'wv8': np.ascontiguousarray(wv8).astype(ml_dtypes.float8_e4m3),
        'wfc18': np.ascontiguousarray(wfc18).astype(ml_dtypes.float8_e4m3),
        'wlep8': np.ascontiguousarray(wlep8).astype(ml_dtypes.float8_e4m3),
        'wprojh': np.ascontiguousarray(wprojh).astype(f16),
        'projb': np.ascontiguousarray(projb.reshape(128, 1)),
        'wfc1': wfc1.astype(f16),
        'fc1b': np.ascontiguousarray(fc1b.reshape(4, 128).T),
        'wfc2': np.ascontiguousarray(
            (wfc2.reshape(4, 128, 128).transpose(1, 0, 2).reshape(128, 512))
            * 16.0).astype(__import__('ml_dtypes').float8_e4m3),
        'fc2b': np.ascontiguousarray(fc2b.reshape(128, 1)),
    }


def build_program(stop_after=None):
    import concourse.tile as tile
    from concourse import bacc, mybir

    dt = mybir.dt

    nc = bacc.Bacc("TRN2", target_bir_lowering=False, debug=False,
                   num_devices=NCORES)

    xin = nc.dram_tensor("xin", [NT, DIM], dt.float32, kind="ExternalInput").ap()
    wd = {}
    for name, shape, d in [
            ("wqkv", [128, 384], dt.float16),
            ("wqk8", [64, 512], dt.float8e4),
            ("wv8", [64, 128], dt.float8e4),
            ("wfc18", [64, 1024], dt.float8e4),
            ("wlep8", [128, 2816], dt.float8e4),
            ("wprojh", [128, 256], dt.float16), ("projb", [128, 1], dt.float32),
            ("wfc1", [128, 512], dt.float16), ("fc1b", [128, 4], dt.float32),
            ("wfc2", [128, 512], dt.float8e4),
            ("fc2b", [128, 1], dt.float32)]:
        wd[name] = nc.dram_tensor(name, shape, d, kind="ExternalInput").ap()
    out_d = nc.dram_tensor("out", [NT, DIM], dt.float32, kind="ExternalOutput").ap()

    with tile.TileContext(nc) as tc:
        _body(tc, mybir, xin, out_d, wd, stop_after=stop_after)
    nc.compile()
    return nc


def _body(tc, mybir, xin, out_d, wd, stop_after=None):
    nc = tc.nc
    dt = mybir.dt
    AF = mybir.ActivationFunctionType
    OP = mybir.AluOpType
    f16, f32 = dt.float16, dt.float32

    dumped = []

    def dump(t, ncols):
        """debug: DMA first min(ncols,12544) cols of [128,*] tile to out."""
        n = min(ncols, NT) // 128 * 128
        dv = out_d[0:n, :].rearrange("(b p) f -> p b f", p=128)
        nc.gpsimd.dma_start(dv, t[:, 0:n].rearrange("p (b f) -> p b f",
                                                    f=128))
        dumped.append(True)

    STAGES = ["load", "ln1", "t1", "qkv", "v", "lepe", "qk", "exp", "sums",
              "av", "recip", "quads", "proj", "ln2", "t2", "mlp"]
    lim = STAGES.index(stop_after) if stop_after else len(STAGES)

    def go(stage):
        return STAGES.index(stage) <= lim

    # ---------------- persistent pools ----------------
    wpool = tc.alloc_tile_pool(name="weights", bufs=1)
    gpool = tc.alloc_tile_pool(name="globals", bufs=1)

    wqkv = wpool.tile([128, 384], f16)
    wlep8 = wpool.tile([128, 2816], dt.float8e4)
    wprojh = wpool.tile([128, 256], f16)
    projb = wpool.tile([128, 1], f32)
    wfc1 = wpool.tile([128, 512], f16)
    fc1b = wpool.tile([128, 4], f32)
    wfc2 = wpool.tile([128, 512], dt.float8e4)
    fc2b = wpool.tile([128, 1], f32)
    ones_t = wpool.tile([128, 32], f16)
    epsb = wpool.tile([128, 1], f32)
    nc.vector.memset(epsb[:], 1e-5)
    # weight DMAs sequenced after the first image-load chunk on the (single)
    # DMA resource: they are not needed until the first qkv matmul
    with tc.tile_wait_until(0.005):
        for name, t in [("wqkv", wqkv), ("wlep8", wlep8),
                        ("wprojh", wprojh),
                        ("projb", projb), ("wfc1", wfc1), ("fc1b", fc1b),
                        ("wfc2", wfc2), ("fc2b", fc2b)]:
            nc.sync.dma_start(t[:], wd[name])
    nc.vector.memset(ones_t[:], 1.0)

    # token-major global tensors, per-image padded to 25 blocks.
    # xh_tm is a 2-image ring: each slice is written by an LN and read by
    # the immediately following transpose.
    x_tm = gpool.tile([128, IPC * IMGCOLS], f16)
    xh_tm = gpool.tile([128, 2 * IMGCOLS], f16)
    r1_tm = x_tm  # residual accumulates in place

    # fp8 natural-order xhat with 56-col zero pads top/bottom (+extra tail
    # pad so zero-weight padding pairs stay in bounds); 2-image ring,
    # per-image data DMA, pads memset once.
    xq8r = gpool.tile([128, 2 * 3360], dt.float8e4)
    for s in range(2):
        nc.gpsimd.memset(xq8r[:, s * 3360:s * 3360 + 56], 0.0)
        nc.gpsimd.memset(xq8r[:, s * 3360 + 3192:s * 3360 + 3360], 0.0)

    import bass_rust as _br

    def ap_pair(t_ap, off1, off2, n):
        """[128, 2, n] overlapping view: pair elements at cols off1/off2."""
        v = t_ap[:, off1:off1 + n].unsqueeze(1)
        raw = [list(x) for x in v.ap]
        raw[1] = [off2 - off1, 2]
        c = v.copy()
        c.ap = _br.VecI64Pair(raw)
        return c

    ATT_SCALE = 1.0 / 4096.0

    NBLK = IPC * NBI  # 100 token blocks

    # ---------------- per-image load (cast fp32->fp16), chunked ----------
    def load_img_chunk(i, b0, b1):
        src = xin[i * L:(i + 1) * L, :]
        bf = min(b1, 24)
        if bf > b0:
            full = src[b0 * 128:bf * 128, :].rearrange("(b p) f -> p b f", p=128)
            dst = x_tm[:, i * IMGCOLS + b0 * 128:i * IMGCOLS + bf * 128
                       ].rearrange("p (b f) -> p b f", b=bf - b0)
            nc.gpsimd.dma_start(dst, full)
        if b1 == NBI:
            nc.gpsimd.dma_start(
                x_tm[0:64, i * IMGCOLS + 24 * 128:i * IMGCOLS + 25 * 128],
                src[24 * 128:L, :])
            nc.gpsimd.memset(
                x_tm[64:128, i * IMGCOLS + 24 * 128:i * IMGCOLS + 25 * 128], 0.0)

    def layernorm(src_tm, dst_tm, sname, blk0, dst_blk0, chunks,
                  pre_chunk=None, post_chunk=None, newton2=False,
                  split_merge=False, apply_pool=False):
        """token-major LN over NBI blocks at blk0, processed in chunks:
        bn_stats per chunk (after pre_chunk), ONE even/odd merge + rsqrt
        (quadratic seed + Newton on DVE), then fused apply per chunk
        followed by post_chunk (e.g. a transpose). split_merge runs the
        whole pipeline per chunk (shorter latency, more DVE ops)."""
        if split_merge:
            for b0, b1 in chunks:
                layernorm(src_tm, dst_tm, f"{sname}_{b0}", blk0, dst_blk0,
                          ((b0, b1),), pre_chunk, post_chunk, newton2,
                          apply_pool=apply_pool)
            return
        nb = NBI
        with tc.tile_pool(name=f"ln_{sname}", bufs=1) as lp:
            st6 = lp.tile([128, nb * 8], f32, name=f"st6_{sname}")
            rs_t = lp.tile([128, nb], f32, name=f"rs_{sname}")
            c_t = lp.tile([128, nb], f32, name=f"c_{sname}")
            mu_t = lp.tile([128, nb], f32, name=f"mu_{sname}")
            m2_t = lp.tile([128, nb], f32, name=f"m2_{sname}")
            for b0, b1 in chunks:
                if pre_chunk is not None:
                    pre_chunk(b0, b1)
                for b in range(b0, b1):
                    nc.vector.bn_stats(st6[:, b * 8:b * 8 + 6],
                                       src_tm[:, (blk0 + b) * 128:
                                              (blk0 + b + 1) * 128])
            c0, c1 = chunks[0][0], chunks[-1][1]
            sp = slice(c0, c1)
            sv = st6.rearrange("p (b s) -> p b s", s=8)
            # stats6 = [cnt_e, mean_e, cnt*var_e, cnt_o, mean_o, cnt*var_o]
            nc.vector.tensor_tensor(mu_t[:, sp], sv[:, sp, 1:2], sv[:, sp, 4:5], OP.add)
            nc.vector.tensor_tensor(c_t[:, sp], sv[:, sp, 1:2], sv[:, sp, 4:5], OP.subtract)
            nc.vector.tensor_tensor(c_t[:, sp], c_t[:, sp], c_t[:, sp], OP.mult)
            nc.vector.tensor_tensor(m2_t[:, sp], sv[:, sp, 2:3], sv[:, sp, 5:6], OP.add)
            nc.vector.scalar_tensor_tensor(m2_t[:, sp], c_t[:, sp], 32.0, m2_t[:, sp],
                                           OP.mult, OP.add)
            t_t = lp.tile([128, nb], f32, name=f"t_{sname}")
            nc.vector.tensor_scalar(m2_t[:, sp], m2_t[:, sp], 1.0 / 128.0, 1e-5,
                                    OP.mult, OP.add)
            nc.vector.tensor_scalar(rs_t[:, sp], m2_t[:, sp], 0.23645927242441878,
                                    -1.0257861053814088, OP.mult, OP.add)
            nc.vector.tensor_tensor(rs_t[:, sp], rs_t[:, sp], m2_t[:, sp], OP.mult)
            nc.vector.tensor_scalar(rs_t[:, sp], rs_t[:, sp], 1.8125565144482214,
                                    None, OP.add)
            for _ in range(2 if newton2 else 1):
                nc.vector.tensor_tensor(t_t[:, sp], rs_t[:, sp], rs_t[:, sp], OP.mult)
                nc.vector.tensor_tensor(t_t[:, sp], t_t[:, sp], m2_t[:, sp], OP.mult)
                nc.vector.tensor_scalar(t_t[:, sp], t_t[:, sp], -0.5, 1.5,
                                        OP.mult, OP.add)
                nc.vector.tensor_tensor(rs_t[:, sp], rs_t[:, sp], t_t[:, sp], OP.mult)
            # c = -(mu_sum/2)*rs
            nc.vector.scalar_tensor_tensor(c_t[:, sp], mu_t[:, sp], -0.5, rs_t[:, sp],
                                           OP.mult, OP.mult)
            aeng = nc.gpsimd if apply_pool else nc.vector
            for b0, b1 in chunks:
                for b in range(b0, b1):
                    aeng.tensor_scalar(
                        dst_tm[:, (dst_blk0 + b) * 128:(dst_blk0 + b + 1) * 128],
                        src_tm[:, (blk0 + b) * 128:(blk0 + b + 1) * 128],
                        rs_t[:, b:b + 1], c_t[:, b:b + 1], OP.mult, OP.add)
                if post_chunk is not None:
                    post_chunk(b0, b1)

    def transpose_chunk(dst, src_tm, slot, b0, b1):
        """xbar DMA transpose of token-major blocks [b0:b1] of ring slot into
        feature-major dst cols [b0*128:b1*128)."""
        nc.sync.dma_start_transpose(
            dst[:, b0 * 128:b1 * 128].rearrange("p (b f) -> p b f", b=b1 - b0),
            src_tm[:, slot * IMGCOLS + b0 * 128:slot * IMGCOLS + b1 * 128])

    # ---------------- fused per-image loop ----------------
    with tc.tile_pool(name="att_fm", bufs=1) as ap, \
         tc.tile_pool(name="att_ps", bufs=1, space="PSUM") as app, \
         tc.tile_pool(name="att_sb", bufs=3) as asb, \
         tc.tile_pool(name="mlp_fm", bufs=1) as mfm, \
         tc.tile_pool(name="mlp_sb", bufs=2) as msb:
        xh_tiles = {}
        xh2_tiles = {}
        mlp_states = {}

        def epi_chunk(i, mlp_f, b0, b1):
            """transpose + residual + store of output blocks [b0:b1).
            Non-tail images add in fp16 and let the (gpsimd) store DMA
            upcast — halves the DVE cost of the residual adds; the last
            image keeps fp32 + HWDGE stores for the shortest drain."""
            nb = b1 - b0
            fin = True
            mlp_t = mfm.tile([128, 1024], f16, name="mlp_t", tag="mlp_t",
                             bufs=2)
            o_sb = mfm.tile([128, 1024], f32, name="o_sb",
                            tag="o_sbF", bufs=2)
            nc.sync.dma_start_transpose(
                mlp_t[:, 0:nb * 128].rearrange("p (b f) -> p b f", b=nb),
                mlp_f[:, b0 * 128:b1 * 128])
            nc.vector.tensor_tensor(
                o_sb[:, 0:nb * 128], mlp_t[:, 0:nb * 128],
                r1_tm[:, i * IMGCOLS + b0 * 128:i * IMGCOLS + b1 * 128],
                OP.add)
            eng = nc.sync if fin else nc.gpsimd
            nfull = min(b1, 24) - b0
            if nfull > 0:
                dst = out_d[i * L + b0 * 128:i * L + (b0 + nfull) * 128, :
                            ].rearrange("(b p) f -> p b f", p=128)
                eng.dma_start(dst, o_sb[:, 0:nfull * 128].rearrange(
                    "p (b f) -> p b f", b=nfull))
            if b1 == 25:
                # tail: blocks 24..24.5 (64 tokens)
                eng.dma_start(out_d[i * L + 24 * 128:(i + 1) * L, :],
                              o_sb[0:64, nfull * 128:nfull * 128 + 128])

        def mlp_start(i):
            xh2 = xh2_tiles.pop(i)
            xh28p = None
            return {
                'i': i,
                'xh2': xh2,
                'xh28p': xh28p,
                'mlp_f': mfm.tile([128, IMGCOLS], f16, name="mlp_f",
                                  tag="mlp_f", bufs=2),
                'h2': {},
            }

        def mlp_fc1(st, t):
            rhs = st['xh2'][:, t * TW:(t + 1) * TW]
            rhs8 = None
            if st['xh28p'] is not None:
                rhs8 = st['xh28p'].rearrange("p (two c) -> p two c", two=2)[
                    :, :, t * TW:(t + 1) * TW]
            h2s = []
            for half in range(2):
                h2 = msb.tile([128, 2 * TW], dt.float8e4,
                              name=f"h2_{half}", tag="hs", bufs=8)
                ps_h = app.tile([128, 1024], f32, name=f"ps_h{half}",
                                tag="pQK", bufs=2)
                for mm in range(2):
                    m = 2 * half + mm
                    if rhs8 is not None:
                        nc.tensor.matmul(
                            ps_h[:, mm * 512:mm * 512 + TW],
                            wfc18[:, m * 256:(m + 1) * 256].rearrange(
                                "p (two c) -> p two c", two=2),
                            rhs8, start=True, stop=True,
                            perf_mode=mybir.MatmulPerfMode.DoubleRow,
                            skip_group_check=True)
                    else:
                        nc.tensor.matmul(ps_h[:, mm * 512:mm * 512 + TW],
                                         wfc1[:, m * 128:(m + 1) * 128],
                                         rhs, start=True, stop=True,
                                         skip_group_check=True)
                nc.scalar.activation(
                    h2.rearrange("p (m c) -> p m c", m=2),
                    ps_h.rearrange("p (m c) -> p m c", m=2)[:, :, 0:TW],
                    AF.Gelu,
                    scale=(1.0 / 32.0) if rhs8 is not None else 1.0)
                h2s.append(h2)
            st['h2'][t] = h2s

        def mlp_fc2(st, t):
            i = st['i']
            mlp_f = st['mlp_f']
            h2s = st['h2'].pop(t)
            ps_o = app.tile([128, 512], f32, name="ps_o", tag="pC", bufs=2)
            # fc2: fp8 DoubleRow pairs two K=128 chunks per pass (weights
            # host-scaled x16; undone in the bias add below)
            for half in range(2):
                nc.tensor.matmul(
                    ps_o[:, 0:TW],
                    wfc2[:, half * 256:(half + 1) * 256].rearrange(
                        "p (two m) -> p two m", two=2),
                    h2s[half].rearrange("p (two c) -> p two c", two=2),
                    start=(half == 0), stop=(half == 1),
                    perf_mode=mybir.MatmulPerfMode.DoubleRow,
                    skip_group_check=True)
            nc.vector.tensor_scalar(mlp_f[:, t * TW:(t + 1) * TW],
                                    ps_o[:, 0:TW], 1.0 / 16.0,
                                    fc2b[:, 0:1], OP.mult, OP.add)
            # stream the epilogue as its input columns land
            if t == 2:
                epi_chunk(i, mlp_f, 0, 8)
            elif t == 4:
                epi_chunk(i, mlp_f, 8, 16)
            elif t == 5:
                epi_chunk(i, mlp_f, 16, 20)
            elif t == 6:
                epi_chunk(i, mlp_f, 20, 24)
                nc.vector.memset(mlp_f[:, L:IMGCOLS], 0.0)
                epi_chunk(i, mlp_f, 24, 25)

        def do_mlp_tail(st):
            if st['xh28p'] is not None:
                xh2 = st['xh2']
                nc.gpsimd.dma_start(st['xh28p'][:, 0:IMGCOLS], xh2[0:64, :])
                nc.gpsimd.dma_start(st['xh28p'][:, IMGCOLS:2 * IMGCOLS],
                                    xh2[64:128, :])
            for t in range(NTILES):
                mlp_fc1(st, t)
                if t >= 1:
                    mlp_fc2(st, t - 1)
            mlp_fc2(st, NTILES - 1)

        def stage_in(i):
            xh = ap.tile([128, IMGCOLS], f16, name="xh_fm", tag="xh", bufs=2)
            slot = i % 2

            def load(b0, b1):
                # keep prefetched images' loads well out of the startup and
                # per-image copy windows: they pollute the rotating DMA
                # semaphores and hog the DMA engines otherwise
                with tc.tile_wait_until(0.05 * (i - 1), enable=i >= 2):
                    load_img_chunk(i, b0, b1)

            layernorm(x_tm, xh_tm, f"ln1_{i}", blk0=NBI * i,
                      dst_blk0=NBI * slot, chunks=((0, 13), (13, NBI)),
                      pre_chunk=load,
                      post_chunk=lambda b0, b1: transpose_chunk(
                          xh, xh_tm, slot, b0, b1),
                      split_merge=(i < 2))
            xh_tiles[i] = xh

        stage_in(0)
        stage_in(1)
        for i in range(IPC):
            xh = xh_tiles[i]
            # fp8 copies of xhat for the DoubleRow lepe taps, issued first:
            # natural-order padded xq8 + the (j-1, j+1) edge-zeroed shifted
            # pair xb1p. (Consumed only at the lepe+proj stage — a full
            # image of slack.)
            xq8 = xq8r[:, (i % 2) * 3360:(i % 2) * 3360 + 3360]
            xb1p = ap.tile([128, 2 * L], dt.float8e4, name="xb1p", bufs=2)
            fp16_img = True
            nc.gpsimd.dma_start(xq8[:, 56:3192], xh[:, 0:L])
            nc.gpsimd.memset(xb1p[:, 0:1], 0.0)
            nc.gpsimd.memset(xb1p[:, 2 * L - 1:2 * L], 0.0)
            nc.gpsimd.dma_start(xb1p[:, 1:L], xh[:, 0:L - 1])
            nc.gpsimd.dma_start(xb1p[:, L:2 * L - 1], xh[:, 1:L])
            xb1e = xb1p.rearrange("p (c i j) -> p c i j", c=2, j=RESO)
            nc.gpsimd.memset(xb1e[:, 0, :, 0:1], 0.0)
            nc.gpsimd.memset(xb1e[:, 1, :, 55:56], 0.0)
            # q/k storage: rows 0:64 (branch0) in window-major cols
            # (112w + 2i + jj), rows 64:128 (branch1) natural (= window-major)
            qk_wm = ap.tile([128, 2 * L], f16, name="qk_wm", bufs=2)
            q_wm = qk_wm[:, 0:L]
            k_wm = qk_wm[:, L:2 * L]
            xh_b0wm = ap.tile([128, L], f16, name="xh_b0wm", bufs=1)
            v_wm = ap.tile([128, 2 * NW * 64], f16, name="v_wm", bufs=1)
            o_wm = ap.tile([128, L], f16, name="o_wm")
            att_f = ap.tile([128, IMGCOLS], f16, name="att_f")

            # qkv: q,k. branch0 halves computed directly in window-major
            # order by a window-ordered moving-operand gather; branch1
            # window-major == natural. One paired evac per tile.
            xq0 = xh[:, 0:L].rearrange("p (i w jj) -> p w i jj", w=NW, jj=2)
            qkv = qk_wm.rearrange("p (qk c) -> p qk c", qk=2)
            for t in range(NTILES) if go("qkv") else []:
                ps_qk = app.tile([128, 1024], f32, name="ps_qk", tag="pQ2",
                                 bufs=1)
                ps_q = ps_qk[:, 0:512]
                ps_k = ps_qk[:, 512:1024]
                DRm = mybir.MatmulPerfMode.DoubleRow
                rhs_wm = xq0[:, 4 * t:4 * t + 4, :, :]
                if fp16_img:
                    # startup/tail image: fp16 branch1 (no xh8p dependency)
                    rhs_nat = xh[:, t * TW:(t + 1) * TW]
                    nc.tensor.matmul(ps_q[64:128, 0:TW], wqkv[:, 64:128],
                                     rhs_nat, start=True, stop=True,
                                     tile_position=(0, 64),
                                     skip_group_check=True)
                    nc.tensor.matmul(ps_q[0:64, 0:TW], wqkv[:, 0:64], rhs_wm,
                                     start=True, stop=True,
                                     tile_position=(0, 0),
                                     skip_group_check=True)
                    nc.tensor.matmul(ps_k[64:128, 0:TW], wqkv[:, 192:256],
                                     rhs_nat, start=True, stop=True,
                                     tile_position=(0, 64),
                                     skip_group_check=True)
                    nc.tensor.matmul(ps_k[0:64, 0:TW], wqkv[:, 128:192],
                                     rhs_wm, start=True, stop=True,
                                     tile_position=(0, 0),
                                     skip_group_check=True)
                else:
                    rhs_nat8 = xh8p.rearrange("p (two c) -> p two c", two=2)[
                        :, :, t * TW:(t + 1) * TW]
                    # branch1 halves via fp8 DoubleRow (zero-padded to dst
                    # base 0, writes all 128 rows), then branch0 fp16 accums
                    nc.tensor.matmul(ps_q[:, 0:TW],
                                     wqk8[:, 0:256].rearrange(
                                         "p (two m) -> p two m", two=2),
                                     rhs_nat8, start=True, stop=False,
                                     perf_mode=DRm, tile_position=(0, 0),
                                     skip_group_check=True)
                    nc.tensor.matmul(ps_q[0:64, 0:TW], wqkv[:, 0:64], rhs_wm,
                                     start=False, stop=True,
                                     tile_position=(0, 0),
                                     skip_group_check=True)
                    nc.tensor.matmul(ps_k[:, 0:TW],
                                     wqk8[:, 256:512].rearrange(
                                         "p (two m) -> p two m", two=2),
                                     rhs_nat8, start=True, stop=False,
                                     perf_mode=DRm, tile_position=(0, 0),
                                     skip_group_check=True)
                    nc.tensor.matmul(ps_k[0:64, 0:TW], wqkv[:, 128:192],
                                     rhs_wm, start=False, stop=True,
                                     tile_position=(0, 0),
                                     skip_group_check=True)
                # split evac: k on DVE, q on Act — the single psum buffer
                # frees as fast as the next tile's matmuls need it
                nc.vector.tensor_copy(k_wm[:, t * TW:(t + 1) * TW],
                                      ps_qk[:, 512:512 + TW])
                nc.scalar.copy(q_wm[:, t * TW:(t + 1) * TW],
                               ps_qk[:, 0:TW])

            # window-major xhat copy for branch-0 stationary operands.
            # The last image's copy runs on DVE: Pool's queue is deep with
            # LN applies and fp8 copies right then, and v-b0 would stall.
            geng = nc.gpsimd
            xb0 = xh_b0wm.rearrange("p (w q2 jj) -> p q2 w jj", w=NW, q2=RESO)
            for t in range(NTILES) if go("v") else []:
                geng.tensor_copy(xb0[:, 8 * t:8 * t + 8, :, :],
                                 xh[:, t * TW:(t + 1) * TW])

            # v window-major; branch1 via fp8 DoubleRow (x32 weights,
            # undone at the evac)
            for br in range(2) if go("v") else []:
                for wg in range(NW // 4):
                    ps_v = app.tile([128, 512], f32, name="ps_v", tag="pC",
                                    bufs=2)
                    for wi in range(4):
                        w = wg * 4 + wi
                        if br == 0:
                            nc.tensor.matmul(
                                ps_v[0:WIN, wi * 64:wi * 64 + 64],
                                xh_b0wm[:, WIN * w:WIN * w + WIN],
                                wqkv[:, 256:320], start=True, stop=True)
                        elif fp16_img:
                            nc.tensor.matmul(
                                ps_v[0:WIN, wi * 64:wi * 64 + 64],
                                xh[:, WIN * w:WIN * w + WIN],
                                wqkv[:, 320:384], start=True, stop=True)
                        else:
                            nc.tensor.matmul(
                                ps_v[0:WIN, wi * 64:wi * 64 + 64],
                                xh8v[:, :, WIN * w:WIN * w + WIN],
                                wv8v, start=True, stop=True,
                                perf_mode=mybir.MatmulPerfMode.DoubleRow)
                    if br == 0:
                        nc.scalar.copy(
                            v_wm[0:WIN, (wg * 4) * 64:(wg * 4 + 4) * 64],
                            ps_v[0:WIN, 0:256])
                    elif fp16_img:
                        nc.scalar.copy(
                            v_wm[0:WIN, (NW + wg * 4) * 64:(NW + wg * 4 + 4) * 64],
                            ps_v[0:WIN, 0:256])
                    else:
                        nc.scalar.activation(
                            v_wm[0:WIN, (NW + wg * 4) * 64:(NW + wg * 4 + 4) * 64],
                            ps_v[0:WIN, 0:256], AF.Copy, scale=1.0 / 32.0)

            def lepe_tile(t, ps_p):
                """9-tap depthwise LePE conv x Wproj, natural token order,
                accumulated into the proj psum: 11 fp8 DoubleRow MMs with
                parity masking via strided psum writes."""
                DR = mybir.MatmulPerfMode.DoubleRow
                T0 = TW * t
                b = 56 + T0  # xq8 col of token T0

                def wsl(k):
                    return wlep8[:, k * 256:k * 256 + 256].rearrange(
                        "p (two m) -> p two m", two=2)

                def mm(k, mov, out, start=False):
                    nc.tensor.matmul(out, wsl(k), mov, start=start,
                                     stop=False, perf_mode=DR,
                                     tile_position=(0, 0),
                                     skip_group_check=True)

                psF = ps_p[:, 0:TW]
                psc = ps_p[:, 0:TW].rearrange("p (c s) -> p c s", s=2)
                psE, psO = psc[:, :, 0:1], psc[:, :, 1:2]
                psr = ps_p[:, 0:TW].rearrange("p (r c) -> p r c", c=56)
                psER, psOR = psr[:, 0:8:2, :], psr[:, 1:8:2, :]

                def ev(pair):  # stride-2 moving view (base = first element)
                    return pair.rearrange("p two (c s) -> p two c s",
                                          s=2)[:, :, :, 0:1]

                def er(pair):  # every-other-56-block moving view
                    return pair.rearrange("p two (r c) -> p two r c",
                                          c=56)[:, :, 0:8:2, :]

                # branch 0 (vertical stripes)
                mm(0, ap_pair(xq8, b - 56, b + 56, TW), psF, start=True)
                mm(1, ev(ap_pair(xq8, b - 55, b + 57, TW)), psE)
                mm(2, ev(ap_pair(xq8, b - 56, b + 56, TW)), psO)
                mm(3, ap_pair(xq8, b, b + 112, TW), psF)
                mm(4, ev(ap_pair(xq8, b + 1, b + 113, TW)), psE)
                mm(5, ev(ap_pair(xq8, b, b + 112, TW)), psO)
                # branch 1 (horizontal stripes); xb1p halves: h0[c]=x[c-1]
                # (kj=0 tap), h1[c]=x[c+1] (kj=2), j-edge zeroed
                mm(6, ap_pair(xb1p, T0, T0 + L, TW), psF)
                mm(7, er(ap_pair(xb1p, T0 + 56, T0 + 56 + L, TW)), psER)
                mm(8, er(ap_pair(xb1p, T0, T0 + L, TW)), psOR)
                mm(9, er(ap_pair(xq8, b, b + 56, TW)), psER)
                mm(10, er(ap_pair(xq8, b, b + 56, TW)), psOR)

            # attention quads (window-major q/k: same slice for both branches)
            def win_ap(t_, g, w):
                return t_[32 * g:32 * g + 32, WIN * w:WIN * w + WIN]

            # quad groups of 4 windows; scores of row group g go to their
            # own psum bank: pair (g0,g1) in one 2-bank tile, (g2,g3) in the
            # next; one paired exp per 2-bank tile.
            for wq in (range(NW // 4) if go("qk") else []):
                scp = [app.tile([128, 1024], f32, name=f"scp{h}", tag="pQK",
                                bufs=2) for h in range(2)]
                p_s = [asb.tile([128, 2 * TW], f16, name=f"p_s{h}",
                                tag=f"ps{h}", bufs=3) for h in range(2)]
                for h in range(2):
                    for wi in range(4):
                        w = wq * 4 + wi
                        for g in (2 * h, 2 * h + 1):
                            sc = scp[h][:, 512 * (g % 2):512 * (g % 2) + 512]
                            nc.tensor.matmul(
                                sc[0:WIN, wi * WIN:(wi + 1) * WIN],
                                win_ap(k_wm, g, w), win_ap(q_wm, g, w),
                                start=(wi == 0), stop=True,
                                tile_position=(32 * g, 0),
                                skip_group_check=True)
                    if not go("exp"):
                        nc.vector.tensor_copy(
                            p_s[h][0:WIN, :].rearrange("p (b c) -> p b c", b=2),
                            scp[h].rearrange("p (b c) -> p b c", b=2)
                            [0:WIN, :, 0:TW])
                    else:
                        nc.scalar.activation(
                            p_s[h][0:WIN, :].rearrange("p (b c) -> p b c", b=2),
                            scp[h].rearrange("p (b c) -> p b c", b=2)
                            [0:WIN, :, 0:TW],
                            AF.Exp,
                            scale=HD_SCALE if (h == 0 or i in (0, IPC - 1))
                            else HD_SCALE / 1024.0)

                def psg(g):
                    return p_s[g // 2][:, TW * (g % 2):TW * (g % 2) + TW]

                # 448-wide sums (4 MMs) + per-window AV (16 MMs), one divide
                ps_sum = app.tile([128, 512], f32, name="ps_sum", tag="pC",
                                  bufs=2)
                ps_av = app.tile([128, 512], f32, name="ps_av", tag="pC",
                                 bufs=2)
                for g in range(4) if go("sums") else []:
                    nc.tensor.matmul(
                        ps_sum[32 * g:32 * g + 32, 0:TW],
                        ones_t[0:WIN, 0:32], psg(g)[0:WIN, 0:TW],
                        start=True, stop=True, tile_position=(0, 32 * g),
                        skip_group_check=True)
                for wi in range(4):
                    w = wq * 4 + wi
                    for g in range(4) if go("av") else []:
                        br, hd = g // 2, g % 2
                        vc = (br * NW + w) * 64 + 32 * hd
                        nc.tensor.matmul(
                            ps_av[32 * g:32 * g + 32, wi * WIN:(wi + 1) * WIN],
                            v_wm[0:WIN, vc:vc + 32],
                            psg(g)[0:WIN, wi * WIN:(wi + 1) * WIN],
                            start=True, stop=True, tile_position=(0, 32 * g),
                            skip_group_check=True)
                if go("recip"):
                    rcp = asb.tile([128, TW], f32, name="rcp")
                    nc.vector.reciprocal_approx_fast(rcp[:], ps_sum[:, 0:TW])
                if go("quads"):
                    # normalize straight into window-major o (all 128 rows);
                    # proj un-permutes branch 0 below.
                    nc.vector.tensor_tensor(
                        o_wm[:, wq * TW:(wq + 1) * TW],
                        ps_av[:, 0:TW], rcp[:], OP.mult)

            # previous image's MLP tiles interleave 1:1 with the proj
            # tiles: the proj phase is PE-heavy and Act-idle, the pQK psum
            # tag is unused here, and LN2(i-1) finished an image ago — so
            # the gelus hide under lepe+proj matmuls instead of forming a
            # serial block at the iteration end.
            mst = mlp_states.pop(i - 1, None) if go("mlp") else None

            # proj: lepe taps (fp8, proj-folded) accumulate into ps_p first,
            # then branch-0 rows re-ordered to natural by a strided moving
            # view (K=64) and branch-1 window-major == natural (K=64).
            ow_b0 = o_wm.rearrange("p (w i jj) -> p i w jj",
                                   w=NW, jj=2)
            for t in range(NTILES) if go("proj") else []:
                if mst is not None:
                    mlp_fc1(mst, t)
                    if t >= 1:
                        mlp_fc2(mst, t - 1)
                ps_p = app.tile([128, 512], f32, name="ps_p", tag="pC",
                                bufs=2)
                if go("lepe"):
                    lepe_tile(t, ps_p)
                nc.tensor.matmul(ps_p[:, 0:TW], wprojh[:, 0:128],
                                 ow_b0[:, 8 * t:8 * t + 8, :, :],
                                 start=not go("lepe"), stop=False,
                                 tile_position=(0, 0), skip_group_check=True)
                nc.tensor.matmul(ps_p[:, 0:TW], wprojh[:, 128:256],
                                 o_wm[:, t * TW:(t + 1) * TW],
                                 start=False, stop=True,
                                 tile_position=(0, 0), skip_group_check=True)
                nc.vector.tensor_scalar(att_f[:, t * TW:(t + 1) * TW],
                                        ps_p[:, 0:TW], ATT_SCALE,
                                        projb[:, 0:1], OP.mult, OP.add)

            if mst is not None:
                mlp_fc2(mst, NTILES - 1)

            # transpose attened to token-major (one xbar DMA), add residual
            if go("proj"):
                att_t = ap.tile([128, IMGCOLS], f16, name="att_t", tag="att_t",
                                bufs=3)
                nc.vector.memset(att_f[:, L:IMGCOLS], 0.0)
                rchunks = (((0, 7), (7, 13), (13, 19), (19, NBI))
                           if i == IPC - 1 else ((0, 13), (13, NBI)))
                for b0, b1 in rchunks:
                    nc.sync.dma_start_transpose(
                        att_t[:, b0 * 128:b1 * 128].rearrange(
                            "p (b f) -> p b f", b=b1 - b0),
                        att_f[:, b0 * 128:b1 * 128])
                    nc.vector.tensor_tensor(
                        r1_tm[:, i * IMGCOLS + b0 * 128:
                              i * IMGCOLS + b1 * 128],
                        att_t[:, b0 * 128:b1 * 128],
                        x_tm[:, i * IMGCOLS + b0 * 128:
                             i * IMGCOLS + b1 * 128],
                        OP.add)

            if stop_after in ("qkv", "v", "lepe", "qk", "exp", "sums",
                              "av", "recip", "quads") and i == 0:
                dbg = {"qkv": q_wm, "v": v_wm}.get(
                    stop_after, o_wm)
                dump(dbg, 3072)

            if i + 2 < IPC:
                stage_in(i + 2)

            # ---- per-image LN2 (overlaps the next image's attention) ----
            if go("ln2"):
                xh2 = mfm.tile([128, IMGCOLS], f16, name="xh2_fm", tag="xh2",
                               bufs=2)
                slot = i % 2
                layernorm(r1_tm, xh_tm, f"ln2_{i}", blk0=NBI * i,
                          dst_blk0=NBI * slot,
                          chunks=(((0, 7), (7, 13), (13, 19), (19, NBI))
                                  if i == IPC - 1 else ((0, 13), (13, NBI))),
                          post_chunk=(lambda b0, b1: transpose_chunk(
                              xh2, xh_tm, slot, b0, b1)) if go("t2") else None,
                          split_merge=(i == IPC - 1))
                xh2_tiles[i] = xh2
                if go("mlp"):
                    mlp_states[i] = mlp_start(i)
        if go("mlp"):
            do_mlp_tail(mlp_states.pop(IPC - 1))

    if stop_after == "load":
        dump(x_tm, NT)
    if stop_after == "ln1":
        dump(xh_tm, NT)
    if stop_after == "proj":
        dump(r1_tm, NT)

    gpool.release()
    wpool.release()


def kernel(**inputs):
    from concourse.bass_utils import run_bass_kernel_spmd

    if 'nc' not in _CACHE:
        _CACHE['nc'] = build_program()
    nc = _CACHE['nc']

    w = _prep_weights(inputs)
    x = np.asarray(inputs['x'], dtype=np.float32)
    in_maps = []
    for c in range(NCORES):
        m = {'xin': np.ascontiguousarray(
            x[c * IPC:(c + 1) * IPC].reshape(NT, DIM))}
        m.update(w)
        in_maps.append(m)
    res = run_bass_kernel_spmd(nc, in_maps, list(range(NCORES)))
    out = np.empty((B, L, DIM), np.float32)
    for c in range(NCORES):
        out[c * IPC:(c + 1) * IPC] = res.results[c]['out'].reshape(IPC, L, DIM)
    return out


# revision 88
# speedup vs baseline: 1.0911x; 1.0019x over previous
"""CSWin transformer block on 8 Trainium2 NeuronCores.

Data-parallel over batch: 32 images -> 4 images per core. Inside each core a
single fused Bass/Tile program runs, per image: LN1 -> qkv -> cross-shaped
window attention (+LePE depthwise conv) -> proj -> residual -> LN2 -> MLP ->
residual, with the MLP of image i interleaved into image i+1's attention.

Layout strategy:
  - residual stream is token-major fp16 ([128 token partitions, blocks*128
    feature cols]), per-image padded to 25 blocks of 128 tokens.
  - matmul-side activations are feature-major fp16 per image [128 ch, 3136].
  - PE transposes (fp16, via identity) bridge the two.
  - attention works on per-window quads: 4 (branch, head) groups row/col
    packed onto the PE array; softmax normalization deferred to after AV
    using 448-wide column sums (ones matmul) and a single fused divide.
  - LePE = 9 shifted matmuls against host-folded (Wv * tap-weight) matrices,
    accumulated in PSUM in the same window-major layout as the attention
    output; both are pushed through proj as K=64 partial matmuls.
"""

import numpy as np

B, RESO, DIM = 32, 56, 128
L = RESO * RESO            # 3136
NCORES = 8
IPC = B // NCORES          # images per core = 4
NT = IPC * L               # tokens per core = 12544
NBI = 25                   # padded 128-token blocks per image (24.5 -> 25)
IMGCOLS = NBI * 128        # 3200 token-major cols per image
WIN = 112                  # window size (56*2)
NW = 28                    # windows per branch per image
TW = 448                   # feature-major token tile (8 image rows)
NTILES = L // TW           # 7
HD_SCALE = float(32) ** -0.5

_CACHE = {}


def _prep_weights(inputs):
    f32 = np.float32
    f16 = np.float16
    g1 = np.asarray(inputs['norm1_g'], f32)
    b1 = np.asarray(inputs['norm1_b'], f32)
    qkv_w = np.asarray(inputs['qkv_w'], f32)
    wqkv = qkv_w * g1[:, None]
    bqkv = np.asarray(inputs['qkv_b'], f32) + b1 @ qkv_w
    assert np.max(np.abs(bqkv)) == 0.0, "nonzero qkv bias path not emitted"

    # lepe tap-folded v-projections: block (br, tap) -> wqkv_v[:,64br:+64]*w[tap,c]
    wv = wqkv[:, 256:384]
    lepe_w = [np.asarray(inputs['lepe_w0'], f32), np.asarray(inputs['lepe_w1'], f32)]
    wvtap = np.zeros((128, 18 * 64), f32)
    for br in range(2):
        for ki in range(3):
            for kj in range(3):
                tap = ki * 3 + kj
                wvtap[:, (br * 9 + tap) * 64:(br * 9 + tap + 1) * 64] = (
                    wv[:, br * 64:br * 64 + 64] * lepe_w[br][ki, kj, 0, :][None, :])

    wproj = np.asarray(inputs['proj_w'], f32)
    lepe_b = np.concatenate([np.asarray(inputs['lepe_b0'], f32),
                             np.asarray(inputs['lepe_b1'], f32)])
    projb = np.asarray(inputs['proj_b'], f32) + lepe_b @ wproj

    g2 = np.asarray(inputs['norm2_g'], f32)
    b2 = np.asarray(inputs['norm2_b'], f32)
    fc1_w = np.asarray(inputs['fc1_w'], f32)
    wfc1 = fc1_w * g2[:, None]
    fc1b = np.asarray(inputs['fc1_b'], f32) + b2 @ fc1_w
    wfc2 = np.asarray(inputs['fc2_w'], f32)
    fc2b = np.asarray(inputs['fc2_b'], f32)

    # fp8 DoubleRow lepe weights folded through proj, x S8 scale (undone on
    # the att_f evac). Each MM slot is a [128(xhat ch), 2(pair), 128(proj
    # out)] stationary; pair element B may be zero (padding pair).
    import ml_dtypes
    S8 = 4096.0

    def wt(br, ki, kj):
        t = ki * 3 + kj
        return (wvtap[:, (br * 9 + t) * 64:(br * 9 + t + 1) * 64]
                @ wproj[64 * br:64 * br + 64, :]) * S8

    zz = np.zeros((128, 128), f32)
    slots = [
        (wt(0, 0, 1), wt(0, 2, 1)),   # 1  b0 row-pair, dj=0, full
        (wt(0, 0, 2), wt(0, 2, 2)),   # 2  b0 row-pair, dj=+1, even cols
        (wt(0, 0, 0), wt(0, 2, 0)),   # 3  b0 row-pair, dj=-1, odd cols
        (wt(0, 1, 1), zz),            # 4  b0 center, full
        (wt(0, 1, 2), zz),            # 5  b0 kj=+1, even cols
        (wt(0, 1, 0), zz),            # 6  b0 kj=-1, odd cols
        (wt(1, 1, 0), wt(1, 1, 2)),   # 7  b1 dj-pair, dii=0, full
        (wt(1, 2, 0), wt(1, 2, 2)),   # 8  b1 diag, dii=+1, even rows
        (wt(1, 0, 0), wt(1, 0, 2)),   # 9  b1 diag, dii=-1, odd rows
        (wt(1, 1, 1), wt(1, 2, 1)),   # 10 b1 (center,ki=+1), even rows
        (wt(1, 0, 1), wt(1, 1, 1)),   # 11 b1 (ki=-1,center), odd rows
    ]
    wlep8 = np.concatenate([np.concatenate(s, 1) for s in slots], 1)
    assert np.abs(wlep8).max() < 400.0, np.abs(wlep8).max()
    wprojh = np.concatenate([
        np.concatenate([wproj[0:64], np.zeros((64, 128), f32)], 0),
        np.concatenate([np.zeros((64, 128), f32), wproj[64:128]], 0)],
        1) * S8
    assert np.abs(wprojh).max() < 6e4

    # fp8 DoubleRow branch-1 qkv / v / fc1 weights: K=128 split into
    # channel pairs (c, c+64) matching the xh8p paired-activation layout.
    # x32 scale keeps fp8 values out of the subnormal range; undone at the
    # exp (q*k -> /1024), the v evac (/32) and the gelu (/32).
    SQ = 32.0

    def pair_k(w):  # [128, M] -> [64, 2, M] -> [64, 2*M]
        return np.concatenate([w[0:64], w[64:128]], 1)

    wqk8 = np.concatenate([
        np.concatenate([np.zeros((64, 64), f32), wqkv[0:64, 64:128] * SQ,
                        np.zeros((64, 64), f32), wqkv[64:128, 64:128] * SQ],
                       1),
        np.concatenate([np.zeros((64, 64), f32), wqkv[0:64, 192:256] * SQ,
                        np.zeros((64, 64), f32), wqkv[64:128, 192:256] * SQ],
                       1)], 1)  # [64, 512]: q-b1 pad128-pair, k-b1 pad128-pair
    wv8 = pair_k(wqkv[:, 320:384] * SQ)       # [64, 128]
    wfc18 = np.concatenate(                   # [64, 1024], per-chunk pairs
        [pair_k(wfc1[:, m * 128:(m + 1) * 128] * SQ) for m in range(4)], 1)
    for nm, arr in (("wqk8", wqk8), ("wv8", wv8), ("wfc18", wfc18)):
        assert np.abs(arr).max() < 400.0, (nm, np.abs(arr).max())

    return {
        'wqkv': wqkv.astype(f16),
        'wqk8': np.ascontiguousarray(wqk8).astype(ml_dtypes.float8_e4m3),
        'wv8': np.ascontiguousarray(wv8).astype(ml_dtypes.float8_e4m3),
        'wfc18': np.ascontiguousarray(wfc18).astype(ml_dtypes.float8_e4m3),
        'wlep8': np.ascontiguousarray(wlep8).astype(ml_dtypes.float8_e4m3),
        'wprojh': np.ascontiguousarray(wprojh).astype(f16),
        'projb': np.ascontiguousarray(projb.reshape(128, 1)),
        'wfc1': wfc1.astype(f16),
        'fc1b': np.ascontiguousarray(fc1b.reshape(4, 128).T),
        'wfc2': np.ascontiguousarray(
            (wfc2.reshape(4, 128, 128).transpose(1, 0, 2).reshape(128, 512))
            * 16.0).astype(__import__('ml_dtypes').float8_e4m3),
        'fc2b': np.ascontiguousarray(fc2b.reshape(128, 1)),
    }


def build_program(stop_after=None):
    import concourse.tile as tile
    from concourse import bacc, mybir

    dt = mybir.dt

    nc = bacc.Bacc("TRN2", target_bir_lowering=False, debug=False,
                   num_devices=NCORES)

    xin = nc.dram_tensor("xin", [NT, DIM], dt.float32, kind="ExternalInput").ap()
    wd = {}
    for name, shape, d in [
            ("wqkv", [128, 384], dt.float16),
            ("wqk8", [64, 512], dt.float8e4),
            ("wv8", [64, 128], dt.float8e4),
            ("wfc18", [64, 1024], dt.float8e4),
            ("wlep8", [128, 2816], dt.float8e4),
            ("wprojh", [128, 256], dt.float16), ("projb", [128, 1], dt.float32),
            ("wfc1", [128, 512], dt.float16), ("fc1b", [128, 4], dt.float32),
            ("wfc2", [128, 512], dt.float8e4),
            ("fc2b", [128, 1], dt.float32)]:
        wd[name] = nc.dram_tensor(name, shape, d, kind="ExternalInput").ap()
    out_d = nc.dram_tensor("out", [NT, DIM], dt.float32, kind="ExternalOutput").ap()

    with tile.TileContext(nc) as tc:
        _body(tc, mybir, xin, out_d, wd, stop_after=stop_after)
    nc.compile()
    return nc


def _body(tc, mybir, xin, out_d, wd, stop_after=None):
    nc = tc.nc
    dt = mybir.dt
    AF = mybir.ActivationFunctionType
    OP = mybir.AluOpType
    f16, f32 = dt.float16, dt.float32

    dumped = []

    def dump(t, ncols):
        """debug: DMA first min(ncols,12544) cols of [128,*] tile to out."""
        n = min(ncols, NT) // 128 * 128
        dv = out_d[0:n, :].rearrange("(b p) f -> p b f", p=128)
        nc.gpsimd.dma_start(dv, t[:, 0:n].rearrange("p (b f) -> p b f",
                                                    f=128))
        dumped.append(True)

    STAGES = ["load", "ln1", "t1", "qkv", "v", "lepe", "qk", "exp", "sums",
              "av", "recip", "quads", "proj", "ln2", "t2", "mlp"]
    lim = STAGES.index(stop_after) if stop_after else len(STAGES)

    def go(stage):
        return STAGES.index(stage) <= lim

    # ---------------- persistent pools ----------------
    wpool = tc.alloc_tile_pool(name="weights", bufs=1)
    gpool = tc.alloc_tile_pool(name="globals", bufs=1)

    wqkv = wpool.tile([128, 384], f16)
    wlep8 = wpool.tile([128, 2816], dt.float8e4)
    wprojh = wpool.tile([128, 256], f16)
    projb = wpool.tile([128, 1], f32)
    wfc1 = wpool.tile([128, 512], f16)
    fc1b = wpool.tile([128, 4], f32)
    wfc2 = wpool.tile([128, 512], dt.float8e4)
    fc2b = wpool.tile([128, 1], f32)
    ones_t = wpool.tile([128, 32], f16)
    epsb = wpool.tile([128, 1], f32)
    nc.vector.memset(epsb[:], 1e-5)
    # weight DMAs sequenced after the first image-load chunk on the (single)
    # DMA resource: they are not needed until the first qkv matmul
    with tc.tile_wait_until(0.005):
        for name, t in [("wqkv", wqkv), ("wlep8", wlep8),
                        ("wprojh", wprojh),
                        ("projb", projb), ("wfc1", wfc1), ("fc1b", fc1b),
                        ("wfc2", wfc2), ("fc2b", fc2b)]:
            nc.sync.dma_start(t[:], wd[name])
    nc.vector.memset(ones_t[:], 1.0)

    # token-major global tensors, per-image padded to 25 blocks.
    # xh_tm is a 2-image ring: each slice is written by an LN and read by
    # the immediately following transpose.
    x_tm = gpool.tile([128, IPC * IMGCOLS], f16)
    xh_tm = gpool.tile([128, 2 * IMGCOLS], f16)
    r1_tm = x_tm  # residual accumulates in place

    # fp8 natural-order xhat with 56-col zero pads top/bottom (+extra tail
    # pad so zero-weight padding pairs stay in bounds); 2-image ring,
    # per-image data DMA, pads memset once.
    xq8r = gpool.tile([128, 2 * 3360], dt.float8e4)
    for s in range(2):
        nc.gpsimd.memset(xq8r[:, s * 3360:s * 3360 + 56], 0.0)
        nc.gpsimd.memset(xq8r[:, s * 3360 + 3192:s * 3360 + 3360], 0.0)

    import bass_rust as _br

    def ap_pair(t_ap, off1, off2, n):
        """[128, 2, n] overlapping view: pair elements at cols off1/off2."""
        v = t_ap[:, off1:off1 + n].unsqueeze(1)
        raw = [list(x) for x in v.ap]
        raw[1] = [off2 - off1, 2]
        c = v.copy()
        c.ap = _br.VecI64Pair(raw)
        return c

    ATT_SCALE = 1.0 / 4096.0

    NBLK = IPC * NBI  # 100 token blocks

    # ---------------- per-image load (cast fp32->fp16), chunked ----------
    def load_img_chunk(i, b0, b1):
        src = xin[i * L:(i + 1) * L, :]
        bf = min(b1, 24)
        if bf > b0:
            full = src[b0 * 128:bf * 128, :].rearrange("(b p) f -> p b f", p=128)
            dst = x_tm[:, i * IMGCOLS + b0 * 128:i * IMGCOLS + bf * 128
                       ].rearrange("p (b f) -> p b f", b=bf - b0)
            nc.gpsimd.dma_start(dst, full)
        if b1 == NBI:
            nc.gpsimd.dma_start(
                x_tm[0:64, i * IMGCOLS + 24 * 128:i * IMGCOLS + 25 * 128],
                src[24 * 128:L, :])
            nc.gpsimd.memset(
                x_tm[64:128, i * IMGCOLS + 24 * 128:i * IMGCOLS + 25 * 128], 0.0)

    def layernorm(src_tm, dst_tm, sname, blk0, dst_blk0, chunks,
                  pre_chunk=None, post_chunk=None, newton2=False,
                  split_merge=False, apply_pool=False):
        """token-major LN over NBI blocks at blk0, processed in chunks:
        bn_stats per chunk (after pre_chunk), ONE even/odd merge + rsqrt
        (quadratic seed + Newton on DVE), then fused apply per chunk
        followed by post_chunk (e.g. a transpose). split_merge runs the
        whole pipeline per chunk (shorter latency, more DVE ops)."""
        if split_merge:
            for b0, b1 in chunks:
                layernorm(src_tm, dst_tm, f"{sname}_{b0}", blk0, dst_blk0,
                          ((b0, b1),), pre_chunk, post_chunk, newton2,
                          apply_pool=apply_pool)
            return
        nb = NBI
        with tc.tile_pool(name=f"ln_{sname}", bufs=1) as lp:
            st6 = lp.tile([128, nb * 8], f32, name=f"st6_{sname}")
            rs_t = lp.tile([128, nb], f32, name=f"rs_{sname}")
            c_t = lp.tile([128, nb], f32, name=f"c_{sname}")
            mu_t = lp.tile([128, nb], f32, name=f"mu_{sname}")
            m2_t = lp.tile([128, nb], f32, name=f"m2_{sname}")
            for b0, b1 in chunks:
                if pre_chunk is not None:
                    pre_chunk(b0, b1)
                for b in range(b0, b1):
                    nc.vector.bn_stats(st6[:, b * 8:b * 8 + 6],
                                       src_tm[:, (blk0 + b) * 128:
                                              (blk0 + b + 1) * 128])
            c0, c1 = chunks[0][0], chunks[-1][1]
            sp = slice(c0, c1)
            sv = st6.rearrange("p (b s) -> p b s", s=8)
            # stats6 = [cnt_e, mean_e, cnt*var_e, cnt_o, mean_o, cnt*var_o]
            nc.vector.tensor_tensor(mu_t[:, sp], sv[:, sp, 1:2], sv[:, sp, 4:5], OP.add)
            nc.vector.tensor_tensor(c_t[:, sp], sv[:, sp, 1:2], sv[:, sp, 4:5], OP.subtract)
            nc.vector.tensor_tensor(c_t[:, sp], c_t[:, sp], c_t[:, sp], OP.mult)
            nc.vector.tensor_tensor(m2_t[:, sp], sv[:, sp, 2:3], sv[:, sp, 5:6], OP.add)
            nc.vector.scalar_tensor_tensor(m2_t[:, sp], c_t[:, sp], 32.0, m2_t[:, sp],
                                           OP.mult, OP.add)
            t_t = lp.tile([128, nb], f32, name=f"t_{sname}")
            nc.vector.tensor_scalar(m2_t[:, sp], m2_t[:, sp], 1.0 / 128.0, 1e-5,
                                    OP.mult, OP.add)
            nc.vector.tensor_scalar(rs_t[:, sp], m2_t[:, sp], 0.23645927242441878,
                                    -1.0257861053814088, OP.mult, OP.add)
            nc.vector.tensor_tensor(rs_t[:, sp], rs_t[:, sp], m2_t[:, sp], OP.mult)
            nc.vector.tensor_scalar(rs_t[:, sp], rs_t[:, sp], 1.8125565144482214,
                                    None, OP.add)
            for _ in range(2 if newton2 else 1):
                nc.vector.tensor_tensor(t_t[:, sp], rs_t[:, sp], rs_t[:, sp], OP.mult)
                nc.vector.tensor_tensor(t_t[:, sp], t_t[:, sp], m2_t[:, sp], OP.mult)
                nc.vector.tensor_scalar(t_t[:, sp], t_t[:, sp], -0.5, 1.5,
                                        OP.mult, OP.add)
                nc.vector.tensor_tensor(rs_t[:, sp], rs_t[:, sp], t_t[:, sp], OP.mult)
            # c = -(mu_sum/2)*rs
            nc.vector.scalar_tensor_tensor(c_t[:, sp], mu_t[:, sp], -0.5, rs_t[:, sp],
                                           OP.mult, OP.mult)
            aeng = nc.gpsimd if apply_pool else nc.vector
            for b0, b1 in chunks:
                for b in range(b0, b1):
                    aeng.tensor_scalar(
                        dst_tm[:, (dst_blk0 + b) * 128:(dst_blk0 + b + 1) * 128],
                        src_tm[:, (blk0 + b) * 128:(blk0 + b + 1) * 128],
                        rs_t[:, b:b + 1], c_t[:, b:b + 1], OP.mult, OP.add)
                if post_chunk is not None:
                    post_chunk(b0, b1)

    def transpose_chunk(dst, src_tm, slot, b0, b1):
        """xbar DMA transpose of token-major blocks [b0:b1] of ring slot into
        feature-major dst cols [b0*128:b1*128)."""
        nc.sync.dma_start_transpose(
            dst[:, b0 * 128:b1 * 128].rearrange("p (b f) -> p b f", b=b1 - b0),
            src_tm[:, slot * IMGCOLS + b0 * 128:slot * IMGCOLS + b1 * 128])

    # ---------------- fused per-image loop ----------------
    with tc.tile_pool(name="att_fm", bufs=1) as ap, \
         tc.tile_pool(name="att_ps", bufs=1, space="PSUM") as app, \
         tc.tile_pool(name="att_sb", bufs=3) as asb, \
         tc.tile_pool(name="mlp_fm", bufs=1) as mfm, \
         tc.tile_pool(name="mlp_sb", bufs=2) as msb:
        xh_tiles = {}
        xh2_tiles = {}
        mlp_states = {}

        def epi_chunk(i, mlp_f, b0, b1):
            """transpose + residual + store of output blocks [b0:b1).
            Non-tail images add in fp16 and let the (gpsimd) store DMA
            upcast — halves the DVE cost of the residual adds; the last
            image keeps fp32 + HWDGE stores for the shortest drain."""
            nb = b1 - b0
            fin = True
            mlp_t = mfm.tile([128, 1024], f16, name="mlp_t", tag="mlp_t",
                             bufs=2)
            o_sb = mfm.tile([128, 1024], f32, name="o_sb",
                            tag="o_sbF", bufs=2)
            nc.sync.dma_start_transpose(
                mlp_t[:, 0:nb * 128].rearrange("p (b f) -> p b f", b=nb),
                mlp_f[:, b0 * 128:b1 * 128])
            nc.vector.tensor_tensor(
                o_sb[:, 0:nb * 128], mlp_t[:, 0:nb * 128],
                r1_tm[:, i * IMGCOLS + b0 * 128:i * IMGCOLS + b1 * 128],
                OP.add)
            eng = nc.sync if fin else nc.gpsimd
            nfull = min(b1, 24) - b0
            if nfull > 0:
                dst = out_d[i * L + b0 * 128:i * L + (b0 + nfull) * 128, :
                            ].rearrange("(b p) f -> p b f", p=128)
                eng.dma_start(dst, o_sb[:, 0:nfull * 128].rearrange(
                    "p (b f) -> p b f", b=nfull))
            if b1 == 25:
                # tail: blocks 24..24.5 (64 tokens)
                eng.dma_start(out_d[i * L + 24 * 128:(i + 1) * L, :],
                              o_sb[0:64, nfull * 128:nfull * 128 + 128])

        def mlp_start(i):
            xh2 = xh2_tiles.pop(i)
            xh28p = None
            return {
                'i': i,
                'xh2': xh2,
                'xh28p': xh28p,
                'mlp_f': mfm.tile([128, IMGCOLS], f16, name="mlp_f",
                                  tag="mlp_f", bufs=2),
                'h2': {},
            }

        def mlp_fc1(st, t):
            rhs = st['xh2'][:, t * TW:(t + 1) * TW]
            rhs8 = None
            if st['xh28p'] is not None:
                rhs8 = st['xh28p'].rearrange("p (two c) -> p two c", two=2)[
                    :, :, t * TW:(t + 1) * TW]
            h2s = []
            for half in range(2):
                h2 = msb.tile([128, 2 * TW], dt.float8e4,
                              name=f"h2_{half}", tag="hs", bufs=8)
                ps_h = app.tile([128, 1024], f32, name=f"ps_h{half}",
                                tag="pQK", bufs=2)
                for mm in range(2):
                    m = 2 * half + mm
                    if rhs8 is not None:
                        nc.tensor.matmul(
                            ps_h[:, mm * 512:mm * 512 + TW],
                            wfc18[:, m * 256:(m + 1) * 256].rearrange(
                                "p (two c) -> p two c", two=2),
                            rhs8, start=True, stop=True,
                            perf_mode=mybir.MatmulPerfMode.DoubleRow,
                            skip_group_check=True)
                    else:
                        nc.tensor.matmul(ps_h[:, mm * 512:mm * 512 + TW],
                                         wfc1[:, m * 128:(m + 1) * 128],
                                         rhs, start=True, stop=True,
                                         skip_group_check=True)
                nc.scalar.activation(
                    h2.rearrange("p (m c) -> p m c", m=2),
                    ps_h.rearrange("p (m c) -> p m c", m=2)[:, :, 0:TW],
                    AF.Gelu,
                    scale=(1.0 / 32.0) if rhs8 is not None else 1.0)
                h2s.append(h2)
            st['h2'][t] = h2s

        def mlp_fc2(st, t):
            i = st['i']
            mlp_f = st['mlp_f']
            h2s = st['h2'].pop(t)
            ps_o = app.tile([128, 512], f32, name="ps_o", tag="pC", bufs=2)
            # fc2: fp8 DoubleRow pairs two K=128 chunks per pass (weights
            # host-scaled x16; undone in the bias add below)
            for half in range(2):
                nc.tensor.matmul(
                    ps_o[:, 0:TW],
                    wfc2[:, half * 256:(half + 1) * 256].rearrange(
                        "p (two m) -> p two m", two=2),
                    h2s[half].rearrange("p (two c) -> p two c", two=2),
                    start=(half == 0), stop=(half == 1),
                    perf_mode=mybir.MatmulPerfMode.DoubleRow,
                    skip_group_check=True)
            nc.vector.tensor_scalar(mlp_f[:, t * TW:(t + 1) * TW],
                                    ps_o[:, 0:TW], 1.0 / 16.0,
                                    fc2b[:, 0:1], OP.mult, OP.add)
            # stream the epilogue as its input columns land
            if t == 2:
                epi_chunk(i, mlp_f, 0, 8)
            elif t == 4:
                epi_chunk(i, mlp_f, 8, 16)
            elif t == 5:
                epi_chunk(i, mlp_f, 16, 20)
            elif t == 6:
                epi_chunk(i, mlp_f, 20, 24)
                nc.vector.memset(mlp_f[:, L:IMGCOLS], 0.0)
                epi_chunk(i, mlp_f, 24, 25)

        def do_mlp_tail(st):
            if st['xh28p'] is not None:
                xh2 = st['xh2']
                nc.gpsimd.dma_start(st['xh28p'][:, 0:IMGCOLS], xh2[0:64, :])
                nc.gpsimd.dma_start(st['xh28p'][:, IMGCOLS:2 * IMGCOLS],
                                    xh2[64:128, :])
            for t in range(NTILES):
                mlp_fc1(st, t)
                if t >= 1:
                    mlp_fc2(st, t - 1)
            mlp_fc2(st, NTILES - 1)

        def stage_in(i):
            xh = ap.tile([128, IMGCOLS], f16, name="xh_fm", tag="xh", bufs=2)
            slot = i % 2

            def load(b0, b1):
                # keep prefetched images' loads well out of the startup and
                # per-image copy windows: they pollute the rotating DMA
                # semaphores and hog the DMA engines otherwise
                with tc.tile_wait_until(0.05 * (i - 1), enable=i >= 2):
                    load_img_chunk(i, b0, b1)

            layernorm(x_tm, xh_tm, f"ln1_{i}", blk0=NBI * i,
                      dst_blk0=NBI * slot, chunks=((0, 13), (13, NBI)),
                      pre_chunk=load,
                      post_chunk=lambda b0, b1: transpose_chunk(
                          xh, xh_tm, slot, b0, b1),
                      split_merge=(i < 2))
            xh_tiles[i] = xh

        stage_in(0)
        stage_in(1)
        for i in range(IPC):
            xh = xh_tiles[i]
            # fp8 copies of xhat for the DoubleRow lepe taps, issued first:
            # natural-order padded xq8 + the (j-1, j+1) edge-zeroed shifted
            # pair xb1p. (Consumed only at the lepe+proj stage — a full
            # image of slack.)
            xq8 = xq8r[:, (i % 2) * 3360:(i % 2) * 3360 + 3360]
            xb1p = ap.tile([128, 2 * L], dt.float8e4, name="xb1p", bufs=2)
            fp16_img = True
            nc.gpsimd.dma_start(xq8[:, 56:3192], xh[:, 0:L])
            nc.gpsimd.memset(xb1p[:, 0:1], 0.0)
            nc.gpsimd.memset(xb1p[:, 2 * L - 1:2 * L], 0.0)
            nc.gpsimd.dma_start(xb1p[:, 1:L], xh[:, 0:L - 1])
            nc.gpsimd.dma_start(xb1p[:, L:2 * L - 1], xh[:, 1:L])
            xb1e = xb1p.rearrange("p (c i j) -> p c i j", c=2, j=RESO)
            nc.gpsimd.memset(xb1e[:, 0, :, 0:1], 0.0)
            nc.gpsimd.memset(xb1e[:, 1, :, 55:56], 0.0)
            # q/k storage: rows 0:64 (branch0) in window-major cols
            # (112w + 2i + jj), rows 64:128 (branch1) natural (= window-major)
            qk_wm = ap.tile([128, 2 * L], f16, name="qk_wm", bufs=2)
            q_wm = qk_wm[:, 0:L]
            k_wm = qk_wm[:, L:2 * L]
            xh_b0wm = ap.tile([128, L], f16, name="xh_b0wm", bufs=1)
            v_wm = ap.tile([128, 2 * NW * 64], f16, name="v_wm", bufs=1)
            o_wm = ap.tile([128, L], f16, name="o_wm")
            att_f = ap.tile([128, IMGCOLS], f16, name="att_f")

            # qkv: q,k. branch0 halves computed directly in window-major
            # order by a window-ordered moving-operand gather; branch1
            # window-major == natural. One paired evac per tile.
            xq0 = xh[:, 0:L].rearrange("p (i w jj) -> p w i jj", w=NW, jj=2)
            qkv = qk_wm.rearrange("p (qk c) -> p qk c", qk=2)
            for t in range(NTILES) if go("qkv") else []:
                ps_qk = app.tile([128, 1024], f32, name="ps_qk", tag="pQ2",
                                 bufs=1)
                ps_q = ps_qk[:, 0:512]
                ps_k = ps_qk[:, 512:1024]
                DRm = mybir.MatmulPerfMode.DoubleRow
                rhs_wm = xq0[:, 4 * t:4 * t + 4, :, :]
                if fp16_img:
                    # startup/tail image: fp16 branch1 (no xh8p dependency)
                    rhs_nat = xh[:, t * TW:(t + 1) * TW]
                    nc.tensor.matmul(ps_q[64:128, 0:TW], wqkv[:, 64:128],
                                     rhs_nat, start=True, stop=True,
                                     tile_position=(0, 64),
                                     skip_group_check=True)
                    nc.tensor.matmul(ps_q[0:64, 0:TW], wqkv[:, 0:64], rhs_wm,
                                     start=True, stop=True,
                                     tile_position=(0, 0),
                                     skip_group_check=True)
                    nc.tensor.matmul(ps_k[64:128, 0:TW], wqkv[:, 192:256],
                                     rhs_nat, start=True, stop=True,
                                     tile_position=(0, 64),
                                     skip_group_check=True)
                    nc.tensor.matmul(ps_k[0:64, 0:TW], wqkv[:, 128:192],
                                     rhs_wm, start=True, stop=True,
                                     tile_position=(0, 0),
                                     skip_group_check=True)
                else:
                    rhs_nat8 = xh8p.rearrange("p (two c) -> p two c", two=2)[
                        :, :, t * TW:(t + 1) * TW]
                    # branch1 halves via fp8 DoubleRow (zero-padded to dst
                    # base 0, writes all 128 rows), then branch0 fp16 accums
                    nc.tensor.matmul(ps_q[:, 0:TW],
                                     wqk8[:, 0:256].rearrange(
                                         "p (two m) -> p two m", two=2),
                                     rhs_nat8, start=True, stop=False,
                                     perf_mode=DRm, tile_position=(0, 0),
                                     skip_group_check=True)
                    nc.tensor.matmul(ps_q[0:64, 0:TW], wqkv[:, 0:64], rhs_wm,
                                     start=False, stop=True,
                                     tile_position=(0, 0),
                                     skip_group_check=True)
                    nc.tensor.matmul(ps_k[:, 0:TW],
                                     wqk8[:, 256:512].rearrange(
                                         "p (two m) -> p two m", two=2),
                                     rhs_nat8, start=True, stop=False,
                                     perf_mode=DRm, tile_position=(0, 0),
                                     skip_group_check=True)
                    nc.tensor.matmul(ps_k[0:64, 0:TW], wqkv[:, 128:192],
                                     rhs_wm, start=False, stop=True,
                                     tile_position=(0, 0),
                                     skip_group_check=True)
                # split evac: k on DVE, q on Act — the single psum buffer
                # frees as fast as the next tile's matmuls need it
                nc.vector.tensor_copy(k_wm[:, t * TW:(t + 1) * TW],
                                      ps_qk[:, 512:512 + TW])
                nc.scalar.copy(q_wm[:, t * TW:(t + 1) * TW],
                               ps_qk[:, 0:TW])

            # window-major xhat copy for branch-0 stationary operands.
            # The last image's copy runs on DVE: Pool's queue is deep with
            # LN applies and fp8 copies right then, and v-b0 would stall.
            geng = nc.gpsimd
            xb0 = xh_b0wm.rearrange("p (w q2 jj) -> p q2 w jj", w=NW, q2=RESO)
            for t in range(NTILES) if go("v") else []:
                geng.tensor_copy(xb0[:, 8 * t:8 * t + 8, :, :],
                                 xh[:, t * TW:(t + 1) * TW])

            # v window-major; branch1 via fp8 DoubleRow (x32 weights,
            # undone at the evac)
            for br in range(2) if go("v") else []:
                for wg in range(NW // 4):
                    ps_v = app.tile([128, 512], f32, name="ps_v", tag="pC",
                                    bufs=2)
                    for wi in range(4):
                        w = wg * 4 + wi
                        if br == 0:
                            nc.tensor.matmul(
                                ps_v[0:WIN, wi * 64:wi * 64 + 64],
                                xh_b0wm[:, WIN * w:WIN * w + WIN],
                                wqkv[:, 256:320], start=True, stop=True)
                        elif fp16_img:
                            nc.tensor.matmul(
                                ps_v[0:WIN, wi * 64:wi * 64 + 64],
                                xh[:, WIN * w:WIN * w + WIN],
                                wqkv[:, 320:384], start=True, stop=True)
                        else:
                            nc.tensor.matmul(
                                ps_v[0:WIN, wi * 64:wi * 64 + 64],
                                xh8v[:, :, WIN * w:WIN * w + WIN],
                                wv8v, start=True, stop=True,
                                perf_mode=mybir.MatmulPerfMode.DoubleRow)
                    if br == 0:
                        nc.scalar.copy(
                            v_wm[0:WIN, (wg * 4) * 64:(wg * 4 + 4) * 64],
                            ps_v[0:WIN, 0:256])
                    elif fp16_img:
                        nc.scalar.copy(
                            v_wm[0:WIN, (NW + wg * 4) * 64:(NW + wg * 4 + 4) * 64],
                            ps_v[0:WIN, 0:256])
                    else:
                        nc.scalar.activation(
                            v_wm[0:WIN, (NW + wg * 4) * 64:(NW + wg * 4 + 4) * 64],
                            ps_v[0:WIN, 0:256], AF.Copy, scale=1.0 / 32.0)

            def lepe_tile(t, ps_p):
                """9-tap depthwise LePE conv x Wproj, natural token order,
                accumulated into the proj psum: 11 fp8 DoubleRow MMs with
                parity masking via strided psum writes."""
                DR = mybir.MatmulPerfMode.DoubleRow
                T0 = TW * t
                b = 56 + T0  # xq8 col of token T0

                def wsl(k):
                    return wlep8[:, k * 256:k * 256 + 256].rearrange(
                        "p (two m) -> p two m", two=2)

                def mm(k, mov, out, start=False):
                    nc.tensor.matmul(out, wsl(k), mov, start=start,
                                     stop=False, perf_mode=DR,
                                     tile_position=(0, 0),
                                     skip_group_check=True)

                psF = ps_p[:, 0:TW]
                psc = ps_p[:, 0:TW].rearrange("p (c s) -> p c s", s=2)
                psE, psO = psc[:, :, 0:1], psc[:, :, 1:2]
                psr = ps_p[:, 0:TW].rearrange("p (r c) -> p r c", c=56)
                psER, psOR = psr[:, 0:8:2, :], psr[:, 1:8:2, :]

                def ev(pair):  # stride-2 moving view (base = first element)
                    return pair.rearrange("p two (c s) -> p two c s",
                                          s=2)[:, :, :, 0:1]

                def er(pair):  # every-other-56-block moving view
                    return pair.rearrange("p two (r c) -> p two r c",
                                          c=56)[:, :, 0:8:2, :]

                # branch 0 (vertical stripes)
                mm(0, ap_pair(xq8, b - 56, b + 56, TW), psF, start=True)
                mm(1, ev(ap_pair(xq8, b - 55, b + 57, TW)), psE)
                mm(2, ev(ap_pair(xq8, b - 56, b + 56, TW)), psO)
                mm(3, ap_pair(xq8, b, b + 112, TW), psF)
                mm(4, ev(ap_pair(xq8, b + 1, b + 113, TW)), psE)
                mm(5, ev(ap_pair(xq8, b, b + 112, TW)), psO)
                # branch 1 (horizontal stripes); xb1p halves: h0[c]=x[c-1]
                # (kj=0 tap), h1[c]=x[c+1] (kj=2), j-edge zeroed
                mm(6, ap_pair(xb1p, T0, T0 + L, TW), psF)
                mm(7, er(ap_pair(xb1p, T0 + 56, T0 + 56 + L, TW)), psER)
                mm(8, er(ap_pair(xb1p, T0, T0 + L, TW)), psOR)
                mm(9, er(ap_pair(xq8, b, b + 56, TW)), psER)
                mm(10, er(ap_pair(xq8, b, b + 56, TW)), psOR)

            # attention quads (window-major q/k: same slice for both branches)
            def win_ap(t_, g, w):
                return t_[32 * g:32 * g + 32, WIN * w:WIN * w + WIN]

            # quad groups of 4 windows; scores of row group g go to their
            # own psum bank: pair (g0,g1) in one 2-bank tile, (g2,g3) in the
            # next; one paired exp per 2-bank tile.
            for wq in (range(NW // 4) if go("qk") else []):
                scp = [app.tile([128, 1024], f32, name=f"scp{h}", tag="pQK",
                                bufs=2) for h in range(2)]
                p_s = [asb.tile([128, 2 * TW], f16, name=f"p_s{h}",
                                tag=f"ps{h}", bufs=3) for h in range(2)]
                for h in range(2):
                    for wi in range(4):
                        w = wq * 4 + wi
                        for g in (2 * h, 2 * h + 1):
                            sc = scp[h][:, 512 * (g % 2):512 * (g % 2) + 512]
                            nc.tensor.matmul(
                                sc[0:WIN, wi * WIN:(wi + 1) * WIN],
                                win_ap(k_wm, g, w), win_ap(q_wm, g, w),
                                start=(wi == 0), stop=True,
                                tile_position=(32 * g, 0),
                                skip_group_check=True)
                    if not go("exp"):
                        nc.vector.tensor_copy(
                            p_s[h][0:WIN, :].rearrange("p (b c) -> p b c", b=2),
                            scp[h].rearrange("p (b c) -> p b c", b=2)
                            [0:WIN, :, 0:TW])
                    else:
                        nc.scalar.activation(
                            p_s[h][0:WIN, :].rearrange("p (b c) -> p b c", b=2),
                            scp[h].rearrange("p (b c) -> p b c", b=2)
                            [0:WIN, :, 0:TW],
                            AF.Exp,
                            scale=HD_SCALE if (h == 0 or i in (0, IPC - 1))
                            else HD_SCALE / 1024.0)

                def psg(g):
                    return p_s[g // 2][:, TW * (g % 2):TW * (g % 2) + TW]

                # 448-wide sums (4 MMs) + per-window AV (16 MMs), one divide
                ps_sum = app.tile([128, 512], f32, name="ps_sum", tag="pC",
                                  bufs=2)
                ps_av = app.tile([128, 512], f32, name="ps_av", tag="pC",
                                 bufs=2)
                for g in range(4) if go("sums") else []:
                    nc.tensor.matmul(
                        ps_sum[32 * g:32 * g + 32, 0:TW],
                        ones_t[0:WIN, 0:32], psg(g)[0:WIN, 0:TW],
                        start=True, stop=True, tile_position=(0, 32 * g),
                        skip_group_check=True)
                for wi in range(4):
                    w = wq * 4 + wi
                    for g in range(4) if go("av") else []:
                        br, hd = g // 2, g % 2
                        vc = (br * NW + w) * 64 + 32 * hd
                        nc.tensor.matmul(
                            ps_av[32 * g:32 * g + 32, wi * WIN:(wi + 1) * WIN],
                            v_wm[0:WIN, vc:vc + 32],
                            psg(g)[0:WIN, wi * WIN:(wi + 1) * WIN],
                            start=True, stop=True, tile_position=(0, 32 * g),
                            skip_group_check=True)
                if go("recip"):
                    rcp = asb.tile([128, TW], f32, name="rcp")
                    nc.vector.reciprocal_approx_fast(rcp[:], ps_sum[:, 0:TW])
                if go("quads"):
                    # normalize straight into window-major o (all 128 rows);
                    # proj un-permutes branch 0 below.
                    nc.vector.tensor_tensor(
                        o_wm[:, wq * TW:(wq + 1) * TW],
                        ps_av[:, 0:TW], rcp[:], OP.mult)

            # previous image's MLP tiles interleave 1:1 with the proj
            # tiles: the proj phase is PE-heavy and Act-idle, the pQK psum
            # tag is unused here, and LN2(i-1) finished an image ago — so
            # the gelus hide under lepe+proj matmuls instead of forming a
            # serial block at the iteration end.
            mst = mlp_states.pop(i - 1, None) if go("mlp") else None

            # proj: lepe taps (fp8, proj-folded) accumulate into ps_p first,
            # then branch-0 rows re-ordered to natural by a strided moving
            # view (K=64) and branch-1 window-major == natural (K=64).
            ow_b0 = o_wm.rearrange("p (w i jj) -> p i w jj",
                                   w=NW, jj=2)
            for t in range(NTILES) if go("proj") else []:
                if mst is not None:
                    mlp_fc1(mst, t)
                    if t >= 1:
                        mlp_fc2(mst, t - 1)
                ps_p = app.tile([128, 512], f32, name="ps_p", tag="pC",
                                bufs=2)
                if go("lepe"):
                    lepe_tile(t, ps_p)
                nc.tensor.matmul(ps_p[:, 0:TW], wprojh[:, 0:128],
                                 ow_b0[:, 8 * t:8 * t + 8, :, :],
                                 start=not go("lepe"), stop=False,
                                 tile_position=(0, 0), skip_group_check=True)
                nc.tensor.matmul(ps_p[:, 0:TW], wprojh[:, 128:256],
                                 o_wm[:, t * TW:(t + 1) * TW],
                                 start=False, stop=True,
                                 tile_position=(0, 0), skip_group_check=True)
                nc.vector.tensor_scalar(att_f[:, t * TW:(t + 1) * TW],
                                        ps_p[:, 0:TW], ATT_SCALE,
                                        projb[:, 0:1], OP.mult, OP.add)

            if mst is not None:
                mlp_fc2(mst, NTILES - 1)

            # transpose attened to token-major (one xbar DMA), add residual
            if go("proj"):
                att_t = ap.tile([128, IMGCOLS], f16, name="att_t", tag="att_t",
                                bufs=3)
                nc.vector.memset(att_f[:, L:IMGCOLS], 0.0)
                rchunks = (((0, 7), (7, 13), (13, 19), (19, NBI))
                           if i == IPC - 1 else ((0, 13), (13, NBI)))
                for b0, b1 in rchunks:
                    nc.sync.dma_start_transpose(
                        att_t[:, b0 * 128:b1 * 128].rearrange(
                            "p (b f) -> p b f", b=b1 - b0),
                        att_f[:, b0 * 128:b1 * 128])
                    nc.vector.tensor_tensor(
                        r1_tm[:, i * IMGCOLS + b0 * 128:
                              i * IMGCOLS + b1 * 128],
                        att_t[:, b0 * 128:b1 * 128],
                        x_tm[:, i * IMGCOLS + b0 * 128:
                             i * IMGCOLS + b1 * 128],
                        OP.add)

            if stop_after in ("qkv", "v", "lepe", "qk", "exp", "sums",
                              "av", "recip", "quads") and i == 0:
                dbg = {"qkv": q_wm, "v": v_wm}.get(
                    stop_after, o_wm)
                dump(dbg, 3072)

            if i + 2 < IPC:
                stage_in(i + 2)

            # ---- per-image LN2 (overlaps the next image's attention) ----
            if go("ln2"):
                xh2 = mfm.tile([128, IMGCOLS], f16, name="xh2_fm", tag="xh2",
                               bufs=2)
                slot = i % 2
                layernorm(r1_tm, xh_tm, f"ln2_{i}", blk0=NBI * i,
                          dst_blk0=NBI * slot,
                          chunks=((0, 13), (13, NBI)),
                          post_chunk=(lambda b0, b1: transpose_chunk(
                              xh2, xh_tm, slot, b0, b1)) if go("t2") else None,
                          split_merge=(i == IPC - 1))
                xh2_tiles[i] = xh2
                if go("mlp"):
                    mlp_states[i] = mlp_start(i)
        if go("mlp"):
            do_mlp_tail(mlp_states.pop(IPC - 1))

    if stop_after == "load":
        dump(x_tm, NT)
    if stop_after == "ln1":
        dump(xh_tm, NT)
    if stop_after == "proj":
        dump(r1_tm, NT)

    gpool.release()
    wpool.release()


def kernel(**inputs):
    from concourse.bass_utils import run_bass_kernel_spmd

    if 'nc' not in _CACHE:
        _CACHE['nc'] = build_program()
    nc = _CACHE['nc']

    w = _prep_weights(inputs)
    x = np.asarray(inputs['x'], dtype=np.float32)
    in_maps = []
    for c in range(NCORES):
        m = {'xin': np.ascontiguousarray(
            x[c * IPC:(c + 1) * IPC].reshape(NT, DIM))}
        m.update(w)
        in_maps.append(m)
    res = run_bass_kernel_spmd(nc, in_maps, list(range(NCORES)))
    out = np.empty((B, L, DIM), np.float32)
    for c in range(NCORES):
        out[c * IPC:(c + 1) * IPC] = res.results[c]['out'].reshape(IPC, L, DIM)
    return out
